# revision 1
# baseline (speedup 1.0000x reference)
"""Bass/Trainium2 SPMD kernel for GQA causal attention with RoPE.

Sharding (8 cores): core c = 4*b + j (b = batch, j = 0..3 shard in batch).
  - Q / attention / o_proj: token-sharded; core j owns q-token 128-blocks
    {j, 7-j, j+8, 15-j} (balanced causal work, uniform SPMD program with
    fixed per-slot key extents [512, 1024, 1536, 2048] and data-driven masks).
  - K and V: token-sharded (core j computes tokens [512j, 512j+512), all
    dims; K roped in (t, d) layout). One fused AllGather per 4-core group
    assembles full K and V; K is PE-transposed on chip to (d, t) tiles.
  - Attention in s^T = (kpos, q) layout: scores = k^T.T @ q^T, exp on ACT
    (scale folds 1/sqrt(hd)), AV with ones-augmented V gives softmax sums,
    division applied after AV (commutes with the linear AV/o_proj steps).
Matmuls run in fp32r (full-rate fp32 on TRN2 at free-dim >= 256); the Q and
output projections use bf16 operands (halves their DMA traffic; verified
end-to-end relative error ~2.7e-3).
"""
import numpy as np

import concourse.bass as bass
import concourse.tile as tile
from concourse import bacc, mybir
from concourse.bass_utils import run_bass_kernel_spmd

B, T, D = 2, 2048, 2048
H, KV, HD = 32, 8, 64
P = 128
NB = T // P          # 16 token blocks of 128
OWN = 4 * P          # 512 owned q tokens per core
f32 = mybir.dt.float32
f32r = mybir.dt.float32r
bf16 = mybir.dt.bfloat16
Exp = mybir.ActivationFunctionType.Exp

KVE = 2 * HD         # 128 kv dims computed per core (2 kv heads)
AG_K = P * T         # 262144 elems of kT shard
AG_V = 512 * 512     # 262144 elems of v shard
AG_N = AG_K + AG_V


def _qblocks(j):
    return [j, 7 - j, j + 8, 15 - j]


def _build(repeat=1, collective=True):
    nc = bacc.Bacc("TRN2", target_bir_lowering=False, debug=False, num_devices=8)

    xtq = nc.dram_tensor("xtq", [D, OWN], bf16, kind="ExternalInput").ap()
    xtv = nc.dram_tensor("xtv", [D, 512], f32, kind="ExternalInput").ap()
    wqt = nc.dram_tensor("wqt", [D, H * HD], bf16, kind="ExternalInput").ap()
    wkt = nc.dram_tensor("wkt", [D, KV * HD], f32, kind="ExternalInput").ap()
    costv8 = nc.dram_tensor("costv8", [512, 512], f32, kind="ExternalInput").ap()
    sintv8 = nc.dram_tensor("sintv8", [512, 512], f32, kind="ExternalInput").ap()
    ident = nc.dram_tensor("ident", [P, P], f32, kind="ExternalInput").ap()
    wvt = nc.dram_tensor("wvt", [D, KV * HD], f32, kind="ExternalInput").ap()
    wot = nc.dram_tensor("wot", [H * HD, D], bf16, kind="ExternalInput").ap()
    costq = nc.dram_tensor("costq", [P, OWN], f32, kind="ExternalInput").ap()
    sintq = nc.dram_tensor("sintq", [P, OWN], f32, kind="ExternalInput").ap()
    masku = nc.dram_tensor("masku", [NB, P, P], bf16, kind="ExternalInput").ap()
    negi = nc.dram_tensor("negi", [P, P], bf16, kind="ExternalInput").ap()
    onesr = nc.dram_tensor("onesr", [1, HD], f32, kind="ExternalInput").ap()
    onesc = nc.dram_tensor("onesc", [P, NB], f32, kind="ExternalInput").ap()
    y = nc.dram_tensor("y", [OWN, D], f32, kind="ExternalOutput").ap()

    def rope_full(dst, src, cosr, sinr, t0, t1, tmp_pool, n):
        """dst[:, t0:t1] = src*cos + swap32(src)*sin_signed over 128 rows.

        cosr rows r = cos[r%64]; sinr rows are sign-folded (-sin for
        (r%64)<32, +sin otherwise). The 32-row half-swaps run on the scalar
        engine (idle during projections); the three full-width elementwise
        ops run on DVE at full lane utilization."""
        xr = tmp_pool.tile([P, 512], f32, tag="xrot", bufs=3, name="xr")
        for po in (0, 64):
            nc.scalar.copy(xr[po:po + 32, :n], src[po + 32:po + 64, :n])
            nc.scalar.copy(xr[po + 32:po + 64, :n], src[po:po + 32, :n])
        u = tmp_pool.tile([P, 512], f32, tag="ropeu", bufs=3, name="u")
        v = tmp_pool.tile([P, 512], f32, tag="ropev", bufs=3, name="v")
        nc.vector.tensor_mul(u[:, :n], src[:, :n], cosr)
        nc.vector.tensor_mul(v[:, :n], xr[:, :n], sinr)
        nc.vector.tensor_add(dst[:, t0:t1], u[:, :n], v[:, :n])

    with tile.TileContext(nc) as tc:
        _dpool_cm = tc.tile_pool(name="dram", bufs=1, space="DRAM")
        dpool = _dpool_cm.__enter__()
        _pers_cm = tc.tile_pool(name="pers", bufs=1)
        pers = _pers_cm.__enter__()

        agin = dpool.tile([AG_N], f32, tag="agin", name="agin")
        agout = dpool.tile([4, AG_N], f32, tag="agout", name="agout")
        obuf = dpool.tile([H * HD, OWN], bf16, tag="obuf", name="obuf")

        # persistent across stages: q cos/sin, qTr, ones
        cosq_t = pers.tile([P, OWN], f32, tag="cosq_t", name="cosq_t")
        sinq_t = pers.tile([P, OWN], f32, tag="sinq_t", name="sinq_t")
        ones_t = pers.tile([1, HD], f32r, tag="ones_t", name="ones_t")
        nc.sync.dma_start(cosq_t[:], costq[:])
        nc.sync.dma_start(sinq_t[:], sintq[:])
        nc.sync.dma_start(ones_t[:], onesr[:].bitcast(f32r))
        qtr = [pers.tile([P, OWN], f32r, tag=f"qtr{i}", name=f"qtr{i}")
               for i in range(16)]

        # ================= stages A-D: projections + AllGather launch ======
        def emit_body():
         with tc.tile_pool(name="consA", bufs=1) as cA, \
             tc.tile_pool(name="wk", bufs=1) as wkp, \
             tc.tile_pool(name="wst", bufs=3) as wst, \
             tc.tile_pool(name="xs", bufs=3) as xsp, \
             tc.tile_pool(name="tmp", bufs=4) as tmpp, \
             tc.tile_pool(name="cpy", bufs=3) as cpyp, \
             tc.tile_pool(name="pproj", bufs=1, space="PSUM") as pproj:


            # ---- stages A+B: token-sharded K and V projections ----
            # rope tables for K in (t, d) layout, head-tiled 8x (per-core rows)
            costd = cA.tile([P, 4 * 512], f32, tag="costd", name="costd")
            sintd = cA.tile([P, 4 * 512], f32, tag="sintd", name="sintd")
            idtA = cA.tile([P, P], f32r, tag="idtA", name="idtA")
            nc.sync.dma_start(idtA[:], ident[:].bitcast(f32r))
            krT_sh = [cA.tile([P, 512], f32, tag=f"krT{db}", name=f"krT{db}")
                      for db in range(4)]
            for vb in range(4):
                nc.sync.dma_start(costd[:, 512 * vb:512 * vb + 512],
                                  costv8[P * vb:P * vb + P, :])
                nc.sync.dma_start(sintd[:, 512 * vb:512 * vb + 512],
                                  sintv8[P * vb:P * vb + P, :])

            psk = [pproj.tile([P, 512], f32, tag=f"k{i}", bufs=1,
                              name=f"psk{i}") for i in range(4)]
            psv = [pproj.tile([P, 512], f32, tag=f"v{i}", bufs=1,
                              name=f"psv{i}") for i in range(4)]
            for cb in range(16):
                wk_t = wst.tile([P, KV * HD], f32r, tag="wk", name="wk")
                nc.sync.dma_start(wk_t[:], wkt[P * cb:P * cb + P, :].bitcast(f32r))
                wv_t = wst.tile([P, KV * HD], f32r, tag="wv", name="wv")
                nc.sync.dma_start(wv_t[:], wvt[P * cb:P * cb + P, :].bitcast(f32r))
                xv_t = xsp.tile([P, 512], f32r, tag="xtv", name="xtv")
                nc.sync.dma_start(
                    xv_t[:], xtv[P * cb:P * cb + P, :].bitcast(f32r))
                for vb in range(4):
                    nc.tensor.matmul(psk[vb][:],
                                     lhsT=xv_t[:, P * vb:P * vb + P],
                                     rhs=wk_t[:],
                                     start=(cb == 0), stop=(cb == 15))
                    nc.tensor.matmul(psv[vb][:],
                                     lhsT=xv_t[:, P * vb:P * vb + P],
                                     rhs=wv_t[:],
                                     start=(cb == 0), stop=(cb == 15))
            # K rope in (t, d) layout; write shard to agin as (512 t, 512 d)
            for vb in range(4):
                cs = costd[:, 512 * vb:512 * vb + 512]
                sn = sintd[:, 512 * vb:512 * vb + 512]
                u = tmpp.tile([P, 512], f32, tag="ropeu", bufs=3, name="u")
                nc.vector.tensor_mul(u[:], psk[vb][:], cs)
                vv = tmpp.tile([P, 512], f32, tag="ropev", bufs=3, name="vv")
                pr = psk[vb].rearrange("p (h s w) -> p h s w", s=2, w=32)
                vr = vv.rearrange("p (h s w) -> p h s w", s=2, w=32)
                sr = sn.rearrange("p (h s w) -> p h s w", s=2, w=32)
                nc.vector.tensor_mul(vr[:, :, 0, :], pr[:, :, 1, :],
                                     sr[:, :, 0, :])
                nc.vector.tensor_mul(vr[:, :, 1, :], pr[:, :, 0, :],
                                     sr[:, :, 1, :])
                kr = cpyp.tile([P, 512], f32r, tag="kr", name="kr")
                nc.vector.tensor_add(kr[:], u[:], vv[:])
                for db in range(4):
                    ptr = pproj.tile([P, P], f32r, tag=f"k{db}", bufs=1,
                                     name="ptrA")
                    nc.tensor.transpose(ptr[:], kr[:, P * db:P * db + P],
                                        idtA[:])
                    if db % 2 == 0:
                        nc.vector.tensor_copy(
                            krT_sh[db][:, P * vb:P * vb + P], ptr[:])
                    else:
                        nc.scalar.copy(
                            krT_sh[db][:, P * vb:P * vb + P], ptr[:])
            for db in range(4):
                nc.sync.dma_start(
                    agin[db * P * 512:(db + 1) * P * 512].rearrange(
                        "(p t) -> p t", t=512), krT_sh[db][:])
            for vb in range(4):
                vs = cpyp.tile([P, 512], f32, tag="vsh", name="vsh")
                nc.scalar.copy(vs[:], psv[vb][:])
                nc.sync.dma_start(
                    agin[AG_K + vb * P * 512:
                         AG_K + (vb + 1) * P * 512].rearrange(
                             "(p t) -> p t", t=512), vs[:])

            # ---- stage C: fused AllGather of (kT shard | v shard) ----
            if collective:
                nc.gpsimd.collective_compute(
                    "AllGather",
                    mybir.AluOpType.bypass,
                    replica_groups=[[0, 1, 2, 3], [4, 5, 6, 7]],
                    ins=[agin.opt()],
                    outs=[agout.opt()],
                )
            else:
                for g in range(4):
                    nc.sync.dma_start(
                        agout[g].rearrange("(a b) -> a b", b=8192),
                        agin.rearrange("(a b) -> a b", b=8192))

            # ---- stage D: Q projection (owned tokens) + rope ----
            xtq_sb = []
            for cb in range(16):
                wt = wkp.tile([P, OWN], bf16, tag=f"xq{cb}", name=f"xq{cb}")
                nc.sync.dma_start(wt[:], xtq[P * cb:P * cb + P, :])
                xtq_sb.append(wt)
            for qg in range(4):
                tg = "k" if qg % 2 == 0 else "v"
                psq = [pproj.tile([P, 512], f32, tag=f"{tg}{i}", bufs=1,
                                  name=f"psq{i}") for i in range(4)]
                for cb in range(16):
                    wq_t = wst.tile([P, 512], bf16, tag="wq", name="wq")
                    nc.sync.dma_start(
                        wq_t[:], wqt[P * cb:P * cb + P,
                                     512 * qg:512 * qg + 512])
                    for qi in range(4):
                        nc.tensor.matmul(
                            psq[qi][:], lhsT=wq_t[:, P * qi:P * qi + P],
                            rhs=xtq_sb[cb], start=(cb == 0), stop=(cb == 15))
                for qi in range(4):
                    rope_full(qtr[4 * qg + qi], psq[qi], cosq_t[:], sinq_t[:],
                              0, OWN, tmpp, OWN)

         # ================= stages E-F: unpack AG + attention ================
         with tc.tile_pool(name="persF", bufs=1) as pF, \
             tc.tile_pool(name="psS", bufs=3, space="PSUM") as psS, \
             tc.tile_pool(name="psO", bufs=2, space="PSUM") as psO, \
             tc.tile_pool(name="ptp", bufs=3) as ptp, \
             tc.tile_pool(name="nrm", bufs=2) as nrm, \
             tc.tile_pool(name="wos", bufs=1) as wos, \
             tc.tile_pool(name="otrg", bufs=1) as otrp, \
             tc.tile_pool(name="psG", bufs=1, space="PSUM") as psG, \
             tc.tile_pool(name="yc", bufs=2) as ycp:

            mask_t = pF.tile([P, NB * P], bf16, tag="mask_t", name="mask_t")
            for kb in range(NB):
                nc.sync.dma_start(mask_t[:, P * kb:P * kb + P], masku[kb])
            negi_t = pF.tile([P, P], bf16, tag="negi_t", name="negi_t")
            nc.sync.dma_start(negi_t[:], negi[:])
            ktr = [pF.tile([P, T], f32r, tag=f"ktr{g}", name=f"ktr{g}")
                   for g in range(4)]
            vaug = [pF.tile([P, NB * (HD + 1)], f32r, tag=f"vaug{kh}",
                            name=f"vaug{kh}") for kh in range(KV)]
            for g in range(4):
                ksrc = agout[g, 0:AG_K].rearrange("(d t) -> d t", t=512)
                for db in range(4):
                    nc.sync.dma_start(
                        ktr[db][:, 512 * g:512 * g + 512],
                        ksrc[P * db:P * db + P, :].bitcast(f32r))
            for kb in range(NB):
                g, lr = kb // 4, (kb % 4) * P
                vsrc = agout[g, AG_K:AG_N].rearrange("(t v) -> t v", v=512)
                vw = ptp.tile([P, 512], f32r, tag="vw", bufs=3, name="vw")
                nc.sync.dma_start(vw[:], vsrc[lr:lr + P, :].bitcast(f32r))
                for kh in range(KV):
                    if kh % 2 == 0:
                        nc.vector.tensor_copy(
                            vaug[kh][:, (HD + 1) * kb:(HD + 1) * kb + HD],
                            vw[:, HD * kh:HD * kh + HD])
                    else:
                        nc.scalar.copy(
                            vaug[kh][:, (HD + 1) * kb:(HD + 1) * kb + HD],
                            vw[:, HD * kh:HD * kh + HD])
            for kh in range(KV):
                ocol = vaug[kh].rearrange("p (k c) -> p k c", c=HD + 1)[:, :, HD]
                nc.sync.dma_start(ocol, onesc[:].bitcast(f32r))

            otr = [None] * 16
            for h in range(H):
                kh = h // 4
                kt, kpo = ktr[kh // 2], HD * (kh % 2)
                # q heads are host-permuted: tile u holds head 8*(u//4)+u%4
                # at rows 0:64 (kv even) and that head +4 at rows 64:128.
                u = (kh // 2) * 4 + h % 4
                qt, qpo = qtr[u], HD * (kh % 2)
                oslot = 2 * u + (kh % 2)
                oaug = psO.tile([P, OWN], f32, tag="oaug", name="oaug")
                for kb in range(NB):
                    qs = P * (kb // 4)
                    n = OWN - qs
                    st = psS.tile([P, OWN], f32, tag="st", bufs=6, name="st")
                    nc.tensor.matmul(
                        st[:, 0:n],
                        lhsT=kt[kpo:kpo + HD, P * kb:P * kb + P],
                        rhs=qt[qpo:qpo + HD, qs:OWN],
                        start=True, stop=True)
                    nc.tensor.matmul(st[:, 0:P], lhsT=negi_t[:],
                                     rhs=mask_t[:, P * kb:P * kb + P],
                                     start=False, stop=True,
                                     skip_group_check=True)
                    pt = ptp.tile([P, OWN], f32r, tag="pt", bufs=6, name="pt")
                    nc.scalar.activation(pt[:, 0:n], st[:, 0:n], Exp, scale=0.125)
                    nc.tensor.matmul(
                        oaug[0:HD + 1, qs:OWN],
                        lhsT=vaug[kh][:, (HD + 1) * kb:(HD + 1) * (kb + 1)],
                        rhs=pt[:, 0:n],
                        start=(kb == 0), stop=(kb == 15))
                rec = nrm.tile([1, OWN], f32, tag="rec", name="rec")
                nc.vector.reciprocal(rec[:], oaug[HD:HD + 1, :])
                recr = nrm.tile([1, OWN], f32r, tag="recr", name="recr")
                nc.vector.tensor_copy(recr[:], rec[:])
                pb = psO.tile([HD, OWN], f32, tag="oaug", name="pb")
                nc.tensor.matmul(pb[:], lhsT=ones_t[:], rhs=recr[:],
                                 start=True, stop=True)
                pbs = nrm.tile([HD, OWN], f32, tag="pbs", bufs=2, name="pbs")
                nc.vector.tensor_copy(pbs[:], pb[:])
                otmp = nrm.tile([HD, OWN], bf16, tag="otmp", bufs=3,
                                name="otmp")
                nc.vector.tensor_mul(otmp[:], oaug[0:HD, :], pbs[:])
                nc.sync.dma_start(obuf[HD * oslot:HD * oslot + HD, :], otmp[:])
                if kh % 2 == 1:
                    wt = otrp.tile([P, OWN], bf16, tag=f"otr{u}",
                                   name=f"otr{u}")
                    nc.sync.dma_start(wt[:], obuf[P * u:P * u + P, :])
                    otr[u] = wt

            # ---- stage G: o_proj (wo chunks prefetch during attention) ----
            wo_ch = {}
            for eb in range(4):
                for ab in range(16):
                    wt = wos.tile([P, 512], bf16, tag=f"wo{ab}",
                                  name=f"wo{eb}_{ab}", bufs=2)
                    nc.sync.dma_start(
                        wt[:], wot[P * ab:P * ab + P,
                                   512 * eb:512 * eb + 512])
                    wo_ch[(eb, ab)] = wt
            for eb in range(4):
                for tb in range(4):
                    # reuse the AV-accumulator banks (free once heads finish)
                    # so o_proj double-buffers without widening the budget
                    psg = psO.tile([P, 512], f32, tag="oaug", name="psg")
                    for ab in range(16):
                        nc.tensor.matmul(
                            psg[:], lhsT=otr[ab][:, P * tb:P * tb + P],
                            rhs=wo_ch[(eb, ab)][:],
                            start=(ab == 0), stop=(ab == 15))
                    yt = ycp.tile([P, 512], f32, tag="yt", name="yt")
                    nc.vector.tensor_copy(yt[:], psg[:])
                    nc.sync.dma_start(
                        y[P * tb:P * tb + P, 512 * eb:512 * eb + 512], yt[:])

        for _rep in range(repeat):
            emit_body()

        _pers_cm.__exit__(None, None, None)
        _dpool_cm.__exit__(None, None, None)

    nc.compile()
    return nc


_NC = None


def _get_nc():
    global _NC
    if _NC is None:
        _NC = _build()
    return _NC


def _head_perm():
    """Pair each even-kv head with its odd-kv partner (+4) in one 128-dim
    block, so q partition parity matches the kv head parity in kT tiles."""
    order = []
    for u in range(16):
        a = 8 * (u // 4) + u % 4
        for h in (a, a + 4):
            order.extend(range(HD * h, HD * h + HD))
    return np.asarray(order)


def _in_maps(x, cos, sin, Wq, Wk, Wv, Wo):
    xT = np.ascontiguousarray(np.transpose(np.asarray(x, np.float32), (0, 2, 1)))
    perm = _head_perm()
    import ml_dtypes as _mld
    WqT = np.ascontiguousarray(
        np.asarray(Wq, np.float32).T[:, perm].astype(_mld.bfloat16))
    WkT = np.ascontiguousarray(np.asarray(Wk, np.float32).T)
    WvT = np.ascontiguousarray(np.asarray(Wv, np.float32).T)
    import ml_dtypes
    WoT = np.ascontiguousarray(
        np.asarray(Wo, np.float32).T[perm, :].astype(ml_dtypes.bfloat16))
    cosT = np.asarray(cos, np.float32).T        # (64, T)
    sinT = np.asarray(sin, np.float32).T
    # 128-row rope tables: row r uses hd-dim r%64; sin rows sign-folded
    # (-sin for (r%64)<32) so rope is x*cos2 + swap32(x)*sin2 on 128 rows.
    sgn = np.where(np.arange(HD) < HD // 2, -1.0, 1.0).astype(np.float32)
    sgnc = sgn  # same sign-fold along the free axis for (t, d) layout
    cos2 = np.ascontiguousarray(np.tile(cosT, (2, 1)))              # (128, T)
    sin2 = np.ascontiguousarray(np.tile(sinT * sgn[:, None], (2, 1)))
    ones = np.ones((1, HD), np.float32)
    maps = []
    for c in range(8):
        b, j = c // 4, c % 4
        qb = _qblocks(j)
        cols = np.concatenate([np.arange(P * g, P * g + P) for g in qb])
        import ml_dtypes
        mask = np.empty((NB, P, P), ml_dtypes.bfloat16)
        ki = np.arange(P)[:, None]
        qi = np.arange(P)[None, :]
        for kb in range(NB):
            qg = qb[kb // 4]
            mask[kb] = np.where(P * kb + ki <= P * qg + qi, 0.0, 1.0)
        negi_np = (np.eye(P, dtype=np.float32) *
                   np.float32(-2.0 ** 30)).astype(ml_dtypes.bfloat16)
        maps.append({
            "xtq": np.ascontiguousarray(
                xT[b][:, cols].astype(_mld.bfloat16)),
            "xtv": np.ascontiguousarray(xT[b][:, 512 * j:512 * j + 512]),
            "wqt": WqT,
            "wkt": WkT,
            "wvt": WvT,
            "wot": WoT,
            "costq": np.ascontiguousarray(cos2[:, cols]),
            "sintq": np.ascontiguousarray(sin2[:, cols]),
            "costv8": np.ascontiguousarray(
                np.tile(cosT.T[512 * j:512 * j + 512, :], (1, KV))),
            "sintv8": np.ascontiguousarray(
                np.tile(sinT.T[512 * j:512 * j + 512, :] * sgnc[None, :],
                        (1, KV))),
            "ident": np.eye(P, dtype=np.float32),
            "masku": mask,
            "negi": negi_np,
            "onesr": ones,
            "onesc": np.ones((P, NB), np.float32),
        })
    return maps


def kernel(x, cos, sin, Wq, Wk, Wv, Wo):
    nc = _get_nc()
    maps = _in_maps(x, cos, sin, Wq, Wk, Wv, Wo)
    res = run_bass_kernel_spmd(nc, maps, list(range(8)))
    out = np.empty((B, T, D), np.float32)
    for c in range(8):
        b, j = c // 4, c % 4
        yc = res.results[c]["y"]
        for s, qg in enumerate(_qblocks(j)):
            out[b, P * qg:P * qg + P, :] = yc[P * s:P * s + P, :]
    return out



# revision 50
# speedup vs baseline: 1137.7133x; 1137.7133x over previous
"""Bass/Trainium2 SPMD kernel for GQA causal attention with RoPE.

Sharding (8 cores): core c = 4*b + j (b = batch, j = 0..3 shard in batch).
  - Q / attention / o_proj: token-sharded; core j owns q-token 128-blocks
    {j, 7-j, j+8, 15-j} (balanced causal work, uniform SPMD program with
    fixed per-slot key extents [512, 1024, 1536, 2048] and data-driven masks).
  - K and V: token-sharded (core j computes tokens [512j, 512j+512), all
    dims; K roped in (t, d) layout). Split AllGathers (K first, then V, both
    bf16) assemble full K and V; K is PE-transposed on chip to (d, t) tiles.
  - Attention in s^T = (kpos, q) layout: scores = k^T.T @ q^T, exp on ACT
    (scale folds 1/sqrt(hd)), AV with ones-augmented V gives softmax sums,
    division applied after AV (commutes with the linear AV/o_proj steps).

v2 structure (cost-model driven):
  - bf16 q/k/v/p operand path: every attention matmul runs 1 cycle/row at
    any free size (fp32r pays 4x below 256), and the K/V AllGather traffic
    halves.
  - exp batching: scores for 2 kb per PSUM group tile ([128,1024] = 2 banks
    for kb 0-7; bank-packed [128,512] tiles for kb 8-15 exploiting the
    PSUM zero-region overwrite semantics) -> 7 exps/head instead of 16.
  - o_proj runs interleaved with attention: after each quad of otr blocks
    completes, its 64 matmuls + DVE adds into bf16 ysb partials are spread
    across the following heads' PE stream (PSUM slots shared with oaug/pb
    via a common pool tag).
  - vaug assembled by 4D-AP DMAs straight from the gathered V (no ACT/DVE
    copies); division output written directly into otr SBUF tiles (no DRAM
    round-trip through obuf).
  - DMA issue split: SP carries the load streams, GPSIMD/SWDGE carries the
    collective-adjacent stores + unpack (25ns issue vs 565ns, and no HWDGE
    contention).
"""
import numpy as np

import concourse.bass as bass
import concourse.tile as tile
from concourse import bacc, mybir
from concourse.bass_utils import run_bass_kernel_spmd

B, T, D = 2, 2048, 2048
H, KV, HD = 32, 8, 64
P = 128
NB = T // P          # 16 token blocks of 128
OWN = 4 * P          # 512 owned q tokens per core
f32 = mybir.dt.float32
f32r = mybir.dt.float32r
bf16 = mybir.dt.bfloat16
Exp = mybir.ActivationFunctionType.Exp
fp8 = mybir.dt.float8e4
DR = mybir.MatmulPerfMode.DoubleRow

AG_K = P * T           # 262144 fp8 elems of folded kT shard
VSH = 4 * KV * (HD + 1)  # 2080 v-shard columns (vaug layout, ones included)
AG_V = P * VSH         # 266240 elems of v shard
VROW = NB * (HD + 1)   # vaug length per kv head in (kb, kh) indexing

# attention group plan: (kbs, kind); kind 2 = [P,1024] two-bank tile with
# one kb per bank, kind 1 = [P,512] single bank packed with 2 or 4 kbs
GROUPS = [((0, 1), 2), ((2, 3), 2), ((4, 5), 2), ((6, 7), 2),
          ((8, 9), 1), ((10, 11), 1), ((12, 13, 14, 15), 1)]


def _qblocks(j):
    return [j, 7 - j, j + 8, 15 - j]


def _build(repeat=1, collective=True):
    nc = bacc.Bacc("TRN2", target_bir_lowering=False, debug=False, num_devices=8)

    xtq = nc.dram_tensor("xtq", [D, OWN], bf16, kind="ExternalInput").ap()
    xtv = nc.dram_tensor("xtv", [D, 512], bf16, kind="ExternalInput").ap()
    wqt = nc.dram_tensor("wqt", [D, H * HD], bf16, kind="ExternalInput").ap()
    wkvt = nc.dram_tensor("wkvt", [D, 2 * KV * HD], bf16,
                          kind="ExternalInput").ap()
    costv8 = nc.dram_tensor("costv8", [512, 512], f32, kind="ExternalInput").ap()
    sintv8 = nc.dram_tensor("sintv8", [512, 512], f32, kind="ExternalInput").ap()
    ident = nc.dram_tensor("ident", [P, P], f32, kind="ExternalInput").ap()
    wot = nc.dram_tensor("wot", [H * HD, D], bf16, kind="ExternalInput").ap()
    costq = nc.dram_tensor("costq", [P, OWN], f32, kind="ExternalInput").ap()
    sintq = nc.dram_tensor("sintq", [P, OWN], f32, kind="ExternalInput").ap()
    masku = nc.dram_tensor("masku", [NB, P, P], bf16, kind="ExternalInput").ap()
    negi = nc.dram_tensor("negi", [P, P], bf16, kind="ExternalInput").ap()
    onesr = nc.dram_tensor("onesr", [1, HD], f32, kind="ExternalInput").ap()
    y = nc.dram_tensor("y", [OWN, D], f32, kind="ExternalOutput").ap()

    def rope_full(dst, src, cosr, sinr, tmp_pool):
        """dst = bf16(src*cos + swap32(src)*sin_signed) over 128 rows.

        cosr rows r = cos[r%64]; sinr rows are sign-folded (-sin for
        (r%64)<32, +sin otherwise). The 32-row half-swaps run on the scalar
        engine (idle during projections); the wide elementwise ops run on
        DVE at full lane utilization."""
        xr = tmp_pool.tile([P, 512], f32, tag="xrot", bufs=3, name="xr")
        for po in (0, 64):
            nc.scalar.copy(xr[po:po + 32, :], src[po + 32:po + 64, :])
            nc.scalar.copy(xr[po + 32:po + 64, :], src[po:po + 32, :])
        u = tmp_pool.tile([P, 512], f32, tag="ropeu", bufs=3, name="u")
        v = tmp_pool.tile([P, 512], f32, tag="ropev", bufs=3, name="v")
        nc.vector.tensor_mul(u[:], src[:], cosr)
        nc.vector.tensor_mul(v[:], xr[:], sinr)
        nc.vector.tensor_add(dst[:], u[:], v[:])

    with tile.TileContext(nc) as tc:
        _dpool_cm = tc.tile_pool(name="dram", bufs=1, space="DRAM")
        dpool = _dpool_cm.__enter__()
        _pers_cm = tc.tile_pool(name="pers", bufs=1)
        pers = _pers_cm.__enter__()

        agink = dpool.tile([AG_K], bf16, tag="agink", name="agink")
        aginv = dpool.tile([AG_V], bf16, tag="aginv", name="aginv")
        agoutk = dpool.tile([4, AG_K], bf16, tag="agoutk", name="agoutk")
        agoutv = dpool.tile([4, AG_V], bf16, tag="agoutv", name="agoutv")

        # persistent across stages
        cosq_t = pers.tile([P, OWN], f32, tag="cosq_t", name="cosq_t")
        sinq_t = pers.tile([P, OWN], f32, tag="sinq_t", name="sinq_t")
        ones_t = pers.tile([1, HD], f32r, tag="ones_t", name="ones_t")
        mask_t = pers.tile([P, NB * P], bf16, tag="mask_t", name="mask_t")
        negi_t = pers.tile([P, P], bf16, tag="negi_t", name="negi_t")
        qtr = [pers.tile([P, OWN], bf16, tag=f"qtr{i}", name=f"qtr{i}")
               for i in range(16)]
        # ktr_all[p, db*2048 + t] = k^T[128*db + p, t]
        ktr_all = pers.tile([P, 4 * T], bf16, tag="ktr_all", name="ktr_all")
        vaug = pers.tile([P, KV * VROW], bf16, tag="vaug", name="vaug")

        def emit_body():
         # ================= stages A-D: projections + AllGather ============
         with tc.tile_pool(name="consA", bufs=1) as cA, \
             tc.tile_pool(name="wk", bufs=1) as wkp, \
             tc.tile_pool(name="wst", bufs=4) as wst, \
             tc.tile_pool(name="xs", bufs=4) as xsp, \
             tc.tile_pool(name="tmp", bufs=4) as tmpp, \
             tc.tile_pool(name="cpy", bufs=3) as cpyp, \
             tc.tile_pool(name="pproj", bufs=1, space="PSUM") as pproj:

            # ---- stages A+B: token-sharded K and V projections ----
            costd = cA.tile([P, 4 * 512], f32, tag="costd", name="costd")
            sintd = cA.tile([P, 4 * 512], f32, tag="sintd", name="sintd")
            idtA = cA.tile([P, P], bf16, tag="idtA", name="idtA")
            idf = cA.tile([P, P], f32, tag="idf", name="idf")
            krT_f = cA.tile([P, 4 * 512], bf16, tag="krT_f", name="krT_f")

            psk = [pproj.tile([P, 512], f32, tag=f"k{i}", bufs=1,
                              name=f"psk{i}") for i in range(4)]
            psv = [pproj.tile([P, 512], f32, tag=f"v{i}", bufs=1,
                              name=f"psv{i}") for i in range(4)]
            xtq_sb = [wkp.tile([P, OWN], bf16, tag=f"xq{cb}", name=f"xq{cb}")
                      for cb in range(16)]
            wqf = [wkp.tile([P, H * HD], bf16, tag=f"wqf{cb}",
                            name=f"wqf{cb}") for cb in range(16)]
            for cb in range(16):
                wkv_t = wst.tile([P, 2 * KV * HD], bf16, tag="wkv",
                                 name="wkv")
                nc.sync.dma_start(wkv_t[:], wkvt[P * cb:P * cb + P, :])
                wk_t = wkv_t[:, 0:KV * HD]
                wv_t = wkv_t[:, KV * HD:2 * KV * HD]
                xv_t = xsp.tile([P, 512], bf16, tag="xtv", name="xtv")
                nc.sync.dma_start(
                    xv_t[:], xtv[P * cb:P * cb + P, :])
                # prefetch streams ride SWDGE (Pool is idle here) so the
                # hot KV loads keep HWDGE to themselves
                nc.gpsimd.dma_start(xtq_sb[cb][:], xtq[P * cb:P * cb + P, :])
                nc.sync.dma_start(wqf[cb][:], wqt[P * cb:P * cb + P, :])
                if cb < 4:
                    nc.gpsimd.dma_start(costd[:, 512 * cb:512 * cb + 512],
                                        costv8[P * cb:P * cb + P, :])
                    nc.gpsimd.dma_start(sintd[:, 512 * cb:512 * cb + 512],
                                        sintv8[P * cb:P * cb + P, :])
                elif cb == 4:
                    nc.gpsimd.dma_start(idf[:], ident[:])
                    nc.scalar.copy(idtA[:], idf[:])
                    nc.gpsimd.dma_start(negi_t[:], negi[:])
                    nc.gpsimd.dma_start(ones_t[:], onesr[:].bitcast(f32r))
                elif cb == 5:
                    nc.gpsimd.dma_start(cosq_t[:], costq[:])
                    nc.gpsimd.dma_start(sinq_t[:], sintq[:])
                elif cb == 6:
                    nc.gpsimd.dma_start(
                        mask_t.rearrange("p (b c) -> p b c", b=NB),
                        masku.rearrange("b p c -> p b c"))
                for vb in range(4):
                    nc.tensor.matmul(psk[vb][:],
                                     lhsT=xv_t[:, P * vb:P * vb + P],
                                     rhs=wk_t,
                                     start=(cb == 0), stop=(cb == 15))
                    nc.tensor.matmul(psv[vb][:],
                                     lhsT=xv_t[:, P * vb:P * vb + P],
                                     rhs=wv_t,
                                     start=(cb == 0), stop=(cb == 15))
            # V shard assembled in vaug layout locally (strided ACT copies +
            # ones columns), then one DMA out
            vs_big = cA.tile([P, VSH], bf16, tag="vs_big", name="vs_big")
            vsv = vs_big.rearrange("p (b k c) -> p b k c", k=KV, c=HD + 1)
            nc.vector.memset(vsv[:, :, :, HD], 1.0)
            for vb in range(4):
                nc.scalar.copy(
                    vsv[:, vb, :, 0:HD],
                    psv[vb].rearrange("p (k c) -> p k c", k=KV))
            nc.gpsimd.dma_start(
                aginv.rearrange("(p c) -> p c", p=P), vs_big[:])
            # K rope in (t, d) layout; transpose to (d, t); write shard
            for vb in range(4):
                cs = costd[:, 512 * vb:512 * vb + 512]
                sn = sintd[:, 512 * vb:512 * vb + 512]
                u = tmpp.tile([P, 512], f32, tag="ropeu", bufs=3, name="u")
                nc.vector.tensor_mul(u[:], psk[vb][:], cs)
                vv = tmpp.tile([P, 512], f32, tag="ropev", bufs=3, name="vv")
                pr = psk[vb].rearrange("p (h s w) -> p h s w", s=2, w=32)
                vr = vv.rearrange("p (h s w) -> p h s w", s=2, w=32)
                sr = sn.rearrange("p (h s w) -> p h s w", s=2, w=32)
                nc.vector.tensor_mul(vr[:, :, 0, :], pr[:, :, 1, :],
                                     sr[:, :, 0, :])
                nc.vector.tensor_mul(vr[:, :, 1, :], pr[:, :, 0, :],
                                     sr[:, :, 1, :])
                kr = cpyp.tile([P, 512], bf16, tag="kr", name="kr")
                nc.vector.tensor_add(kr[:], u[:], vv[:])
                for db in range(4):
                    ptr = pproj.tile([P, P], bf16, tag=f"v{db}", bufs=1,
                                     name="ptrA")
                    nc.tensor.transpose(ptr[:], kr[:, P * db:P * db + P],
                                        idtA[:])
                    dst = krT_f[:, 512 * db + P * vb:512 * db + P * vb + P]
                    if db % 2 == 0:
                        nc.vector.tensor_copy(dst, ptr[:])
                    else:
                        nc.scalar.copy(dst, ptr[:])
            nc.gpsimd.dma_start(
                agink.rearrange("(p c) -> p c", p=P), krT_f[:])

            # ---- stage C: split AllGathers (V first — ready earlier) ----
            if collective:
                nc.gpsimd.collective_compute(
                    "AllGather",
                    mybir.AluOpType.bypass,
                    replica_groups=[[0, 1, 2, 3], [4, 5, 6, 7]],
                    ins=[aginv.opt()],
                    outs=[agoutv.opt()],
                )
                nc.gpsimd.collective_compute(
                    "AllGather",
                    mybir.AluOpType.bypass,
                    replica_groups=[[0, 1, 2, 3], [4, 5, 6, 7]],
                    ins=[agink.opt()],
                    outs=[agoutk.opt()],
                )
            else:
                for g in range(4):
                    nc.gpsimd.dma_start(
                        agoutv[g].rearrange("(a b) -> a b", b=8320),
                        aginv.rearrange("(a b) -> a b", b=8320))
                for g in range(4):
                    nc.gpsimd.dma_start(
                        agoutk[g].rearrange("(a b) -> a b", b=8192),
                        agink.rearrange("(a b) -> a b", b=8192))

            # ---- unpack: both in-gather layouts need 4 DMAs each ----
            for g in range(4):
                ksrc = agoutk[g, 0:AG_K].rearrange(
                    "(p d t) -> p d t", p=P, t=512)
                kdst = ktr_all.rearrange("p (d t) -> p d t", d=4)[
                    :, :, 512 * g:512 * g + 512]
                nc.gpsimd.dma_start(kdst, ksrc)
            for g in range(4):
                nc.gpsimd.dma_start(
                    vaug[:, VSH * g:VSH * g + VSH],
                    agoutv[g, 0:AG_V].rearrange("(p c) -> p c", p=P))

            # ---- stage D: Q projection (owned tokens) + rope ----
            for qg in range(4):
                tg = "k" if qg % 2 == 0 else "v"
                psq = [pproj.tile([P, 512], f32, tag=f"{tg}{i}", bufs=1,
                                  name=f"psq{i}") for i in range(4)]
                for cb in range(16):
                    wq_t = wqf[cb][:, 512 * qg:512 * qg + 512]
                    for qi in range(4):
                        nc.tensor.matmul(
                            psq[qi][:], lhsT=wq_t[:, P * qi:P * qi + P],
                            rhs=xtq_sb[cb], start=(cb == 0), stop=(cb == 15))
                for qi in range(4):
                    rope_full(qtr[4 * qg + qi], psq[qi], cosq_t[:],
                              sinq_t[:], tmpp)

         # ================= stages E-G: attention + fused o_proj ===========
         with tc.tile_pool(name="psS2", bufs=1, space="PSUM") as psS2, \
             tc.tile_pool(name="psS1", bufs=1, space="PSUM") as psS1, \
             tc.tile_pool(name="psO", bufs=1, space="PSUM") as psO, \
             tc.tile_pool(name="ptp", bufs=4) as ptp, \
             tc.tile_pool(name="nrm", bufs=2) as nrm, \
             tc.tile_pool(name="wos", bufs=1) as wos, \
             tc.tile_pool(name="otrg", bufs=1) as otrp, \
             tc.tile_pool(name="ysbp", bufs=1) as ysbp, \
             tc.tile_pool(name="yc", bufs=2) as ycp:

            otr = [otrp.tile([P, OWN], bf16, tag=f"otr{i % 8}",
                             name=f"otr{i}") for i in range(16)]
            ysb = {}
            for eb in range(4):
                for tb in range(4):
                    ysb[(eb, tb)] = ysbp.tile(
                        [P, 512], bf16, tag=f"ysb{eb}_{tb}",
                        name=f"ysb{eb}_{tb}")

            # wo chunk prefetch: o_proj contraction chunk ci covers
            # ab in CHUNKS[ci]; prefetched two chunks ahead.
            CHUNKS = [(0, 4), (4, 8), (8, 12), (12, 15), (15, 16)]
            wo_ch = {}

            def prefetch_chunk(ci):
                for ab in range(*CHUNKS[ci]):
                    for eb in range(4):
                        wt = wos.tile([P, 512], bf16, tag=f"wo{ab % 8}_{eb}",
                                      name=f"wo{ab}_{eb}", bufs=1)
                        nc.sync.dma_start(
                            wt[:], wot[P * ab:P * ab + P,
                                       512 * eb:512 * eb + 512])
                        wo_ch[(eb, ab)] = wt

            prefetch_chunk(0)
            prefetch_chunk(1)

            # o_proj work queue: (q, eb, tb) quanta emitted between attention
            # groups to fill the PE; the PSUM slot tag is picked at pop time
            # (the score tags join the rotation once attention has drained).
            oproj_work = []

            def enqueue_chunk(ci):
                for eb in range(4):
                    for tb in range(4):
                        oproj_work.append((ci, eb, tb))

            def emit_oproj(ci, eb, tb, tag):
                lo, hi = CHUNKS[ci]
                pool = psO if tag == "oaug" else (
                    psS1 if tag == "st1" else psS2)
                psg = pool.tile([P, 512], f32, tag=tag, bufs=2, name="psg")
                for ab in range(lo, hi):
                    nc.tensor.matmul(
                        psg[:], lhsT=otr[ab][:, P * tb:P * tb + P],
                        rhs=wo_ch[(eb, ab)],
                        start=(ab == lo), stop=(ab == hi - 1))
                t = ysb[(eb, tb)]
                if ci == 0:
                    nc.vector.tensor_copy(t[:], psg[:])
                elif ci < len(CHUNKS) - 1:
                    nc.vector.tensor_add(t[:], t[:], psg[:])
                else:
                    yt = ycp.tile([P, 512], f32, tag="yt", bufs=4,
                                  name="yt")
                    nc.vector.tensor_add(yt[:], t[:], psg[:])
                    eng = nc.sync if (eb + tb) % 2 == 0 else nc.gpsimd
                    eng.dma_start(
                        y[P * tb:P * tb + P, 512 * eb:512 * eb + 512], yt[:])

            def pop_oproj(k, tags=("oaug",)):
                for i in range(min(k, len(oproj_work))):
                    ci, eb, tb = oproj_work.pop(0)
                    emit_oproj(ci, eb, tb, tags[i % len(tags)])

            def emit_group(st, pt, kbs, kind, w, v4, m, side):
                qs = P * (kbs[0] // 4)
                n = OWN - qs
                kpo = HD * side
                kt = ktr_all[kpo:kpo + HD,
                             2048 * v4:2048 * v4 + 2048]
                rhs = qtr[4 * v4 + m][kpo:kpo + HD, qs:OWN]
                for i, kb in enumerate(kbs):
                    nc.tensor.matmul(
                        st[:, w * i:w * i + n],
                        lhsT=kt[:, P * kb:P * kb + P],
                        rhs=rhs,
                        start=(kind == 2 or i == 0), stop=False,
                        skip_group_check=True)
                if len(kbs) == 4:
                    nc.tensor.matmul(
                        st[:, 0:512], lhsT=negi_t[:],
                        rhs=mask_t[:, P * kbs[0]:P * kbs[0] + 512],
                        start=False, stop=True, skip_group_check=True)
                else:
                    for i, kb in enumerate(kbs):
                        nc.tensor.matmul(
                            st[:, w * i:w * i + P], lhsT=negi_t[:],
                            rhs=mask_t[:, P * kb:P * kb + P],
                            start=False, stop=(i == len(kbs) - 1),
                            skip_group_check=True)
                # one exp per group (strided across banks for kind 2)
                if kind == 2 and n < 512:
                    stv = st.rearrange("p (s c) -> p s c", s=2)
                    ptv = pt.rearrange("p (s c) -> p s c", s=2)
                    nc.scalar.activation(ptv[:, :, 0:n], stv[:, :, 0:n],
                                         Exp, scale=0.125)
                else:
                    m = w * (len(kbs) - 1) + n
                    nc.scalar.activation(pt[:, 0:m], st[:, 0:m], Exp,
                                         scale=0.125)

            for uu in range(16):
                v4 = uu // 4
                # head pair interleaved at group granularity: tile uu rows
                # 0:64 = head a (kv 2*v4), rows 64:128 = head a+4 (2*v4+1)
                m_u = uu % 4
                oaug = [psO.tile([P, OWN], f32, tag="oaug", bufs=2,
                                 name=f"oaug{s_}") for s_ in range(2)]
                pend = {0: [], 1: []}  # per side, AV lag 2
                for kbs, kind in GROUPS:
                    for side in range(2):
                        kh = 2 * v4 + side
                        if kind == 2:
                            st = psS2.tile([P, 1024], f32, tag="st2",
                                           bufs=2, name="st2")
                            w = 512
                        else:
                            st = psS1.tile([P, 512], f32, tag="st1",
                                           bufs=2, name="st1")
                            w = 512 // len(kbs)
                        pt = ptp.tile([P, 1024], bf16, tag="pt", bufs=8,
                                      name="pt")
                        emit_group(st, pt, kbs, kind, w, v4, m_u, side)
                        pend[side].append((pt, kbs, w))
                        if len(pend[side]) > 2:
                            _emit_avs(nc, pend[side].pop(0), vaug,
                                      oaug[side], kh)
                for side in range(2):
                    for pe_ in pend[side]:
                        _emit_avs(nc, pe_, vaug, oaug[side], 2 * v4 + side)
                for side in range(2):
                    # normalization: rec = 1/sums; Pool broadcasts to 64 rows
                    rec = nrm.tile([1, OWN], f32, tag="rec", name="rec")
                    nc.vector.reciprocal(rec[:], oaug[side][HD:HD + 1, :])
                    pbs = nrm.tile([HD, OWN], f32, tag="pbs", bufs=2,
                                   name="pbs")
                    nc.gpsimd.partition_broadcast(pbs[:], rec[:])
                    nc.vector.tensor_mul(
                        otr[uu][HD * side:HD * side + HD, :],
                        oaug[side][0:HD, :], pbs[:])
                pop_oproj(4, tags=("st1",))
                ci = {3: 0, 7: 1, 11: 2, 13: 3, 15: 4}.get(uu)
                if ci is not None:
                    enqueue_chunk(ci)
                    if ci + 2 < len(CHUNKS):
                        prefetch_chunk(ci + 2)
            pop_oproj(len(oproj_work), tags=("oaug", "st1", "st2"))

        for _rep in range(repeat):
            emit_body()

        _pers_cm.__exit__(None, None, None)
        _dpool_cm.__exit__(None, None, None)

    nc.compile()
    return nc


def _emit_avs(nc, pend, vaug, oaug, kh):
    pt, kbs, w = pend
    qs = P * (kbs[0] // 4)
    n = OWN - qs
    for i, kb in enumerate(kbs):
        base = (HD + 1) * (KV * kb + kh)
        nc.tensor.matmul(
            oaug[0:HD + 1, qs:OWN],
            lhsT=vaug[:, base:base + HD + 1],
            rhs=pt[:, w * i:w * i + n],
            start=(kb == 0), stop=(kb == NB - 1))


_NC = None


def _get_nc():
    global _NC
    if _NC is None:
        _NC = _build()
    return _NC


def _head_perm():
    """Pair each even-kv head with its odd-kv partner (+4) in one 128-dim
    block, so q partition parity matches the kv head parity in kT tiles."""
    order = []
    for u in range(16):
        a = 8 * (u // 4) + u % 4
        for h in (a, a + 4):
            order.extend(range(HD * h, HD * h + HD))
    return np.asarray(order)


def _in_maps(x, cos, sin, Wq, Wk, Wv, Wo):
    xT = np.ascontiguousarray(np.transpose(np.asarray(x, np.float32), (0, 2, 1)))
    perm = _head_perm()
    import ml_dtypes as _mld
    WqT = np.ascontiguousarray(
        np.asarray(Wq, np.float32).T[:, perm].astype(_mld.bfloat16))
    WkvT = np.ascontiguousarray(np.concatenate(
        [np.asarray(Wk, np.float32).T, np.asarray(Wv, np.float32).T],
        axis=1).astype(_mld.bfloat16))
    WoT = np.ascontiguousarray(
        np.asarray(Wo, np.float32).T[perm, :].astype(_mld.bfloat16))
    cosT = np.asarray(cos, np.float32).T        # (64, T)
    sinT = np.asarray(sin, np.float32).T
    # 128-row rope tables: row r uses hd-dim r%64; sin rows sign-folded
    # (-sin for (r%64)<32) so rope is x*cos2 + swap32(x)*sin2 on 128 rows.
    sgn = np.where(np.arange(HD) < HD // 2, -1.0, 1.0).astype(np.float32)
    sgnc = sgn  # same sign-fold along the free axis for (t, d) layout
    cos2 = np.ascontiguousarray(np.tile(cosT, (2, 1)))              # (128, T)
    sin2 = np.ascontiguousarray(np.tile(sinT * sgn[:, None], (2, 1)))
    ones = np.ones((1, HD), np.float32)
    maps = []
    for c in range(8):
        b, j = c // 4, c % 4
        qb = _qblocks(j)
        cols = np.concatenate([np.arange(P * g, P * g + P) for g in qb])
        mask = np.empty((NB, P, P), _mld.bfloat16)
        ki = np.arange(P)[:, None]
        qi = np.arange(P)[None, :]
        for kb in range(NB):
            qg = qb[kb // 4]
            mask[kb] = np.where(P * kb + ki <= P * qg + qi, 0.0, 1.0)
        negi_np = (np.eye(P, dtype=np.float32) *
                   np.float32(-2.0 ** 30)).astype(_mld.bfloat16)
        maps.append({
            "xtq": np.ascontiguousarray(
                xT[b][:, cols].astype(_mld.bfloat16)),
            "xtv": np.ascontiguousarray(
                xT[b][:, 512 * j:512 * j + 512].astype(_mld.bfloat16)),
            "wqt": WqT,
            "wkvt": WkvT,
            "wot": WoT,
            "costq": np.ascontiguousarray(cos2[:, cols]),
            "sintq": np.ascontiguousarray(sin2[:, cols]),
            "costv8": np.ascontiguousarray(
                np.tile(cosT.T[512 * j:512 * j + 512, :], (1, KV))),
            "sintv8": np.ascontiguousarray(
                np.tile(sinT.T[512 * j:512 * j + 512, :] * sgnc[None, :],
                        (1, KV))),
            "ident": np.eye(P, dtype=np.float32),
            "masku": mask,
            "negi": negi_np,
            "onesr": ones,
        })
    return maps


def kernel(x, cos, sin, Wq, Wk, Wv, Wo):
    nc = _get_nc()
    maps = _in_maps(x, cos, sin, Wq, Wk, Wv, Wo)
    res = run_bass_kernel_spmd(nc, maps, list(range(8)))
    out = np.empty((B, T, D), np.float32)
    for c in range(8):
        b, j = c // 4, c % 4
        yc = res.results[c]["y"]
        for s, qg in enumerate(_qblocks(j)):
            out[b, P * qg:P * qg + P, :] = yc[P * s:P * s + P, :]
    return out


# revision 52
# speedup vs baseline: 1147.0170x; 1.0082x over previous
"""Bass/Trainium2 SPMD kernel for GQA causal attention with RoPE.

Sharding (8 cores): core c = 4*b + j (b = batch, j = 0..3 shard in batch).
  - Q / attention / o_proj: token-sharded; core j owns q-token 128-blocks
    {j, 7-j, j+8, 15-j} (balanced causal work, uniform SPMD program with
    fixed per-slot key extents [512, 1024, 1536, 2048] and data-driven masks).
  - K and V: token-sharded (core j computes tokens [512j, 512j+512), all
    dims; K roped in (t, d) layout). Split AllGathers (K first, then V, both
    bf16) assemble full K and V; K is PE-transposed on chip to (d, t) tiles.
  - Attention in s^T = (kpos, q) layout: scores = k^T.T @ q^T, exp on ACT
    (scale folds 1/sqrt(hd)), AV with ones-augmented V gives softmax sums,
    division applied after AV (commutes with the linear AV/o_proj steps).

v2 structure (cost-model driven):
  - bf16 q/k/v/p operand path: every attention matmul runs 1 cycle/row at
    any free size (fp32r pays 4x below 256), and the K/V AllGather traffic
    halves.
  - exp batching: scores for 2 kb per PSUM group tile ([128,1024] = 2 banks
    for kb 0-7; bank-packed [128,512] tiles for kb 8-15 exploiting the
    PSUM zero-region overwrite semantics) -> 7 exps/head instead of 16.
  - o_proj runs interleaved with attention: after each quad of otr blocks
    completes, its 64 matmuls + DVE adds into bf16 ysb partials are spread
    across the following heads' PE stream (PSUM slots shared with oaug/pb
    via a common pool tag).
  - vaug assembled by 4D-AP DMAs straight from the gathered V (no ACT/DVE
    copies); division output written directly into otr SBUF tiles (no DRAM
    round-trip through obuf).
  - DMA issue split: SP carries the load streams, GPSIMD/SWDGE carries the
    collective-adjacent stores + unpack (25ns issue vs 565ns, and no HWDGE
    contention).
"""
import numpy as np

import concourse.bass as bass
import concourse.tile as tile
from concourse import bacc, mybir
from concourse.bass_utils import run_bass_kernel_spmd

B, T, D = 2, 2048, 2048
H, KV, HD = 32, 8, 64
P = 128
NB = T // P          # 16 token blocks of 128
OWN = 4 * P          # 512 owned q tokens per core
f32 = mybir.dt.float32
f32r = mybir.dt.float32r
bf16 = mybir.dt.bfloat16
Exp = mybir.ActivationFunctionType.Exp
fp8 = mybir.dt.float8e4
DR = mybir.MatmulPerfMode.DoubleRow

AG_K = P * T           # 262144 fp8 elems of folded kT shard
VSH = 4 * KV * (HD + 1)  # 2080 v-shard columns (vaug layout, ones included)
AG_V = P * VSH         # 266240 elems of v shard
VROW = NB * (HD + 1)   # vaug length per kv head in (kb, kh) indexing

# attention group plan: (kbs, kind); kind 2 = [P,1024] two-bank tile with
# one kb per bank, kind 1 = [P,512] single bank packed with 2 or 4 kbs
GROUPS = [((0, 1), 2), ((2, 3), 2), ((4, 5), 2), ((6, 7), 2),
          ((8, 9), 1), ((10, 11), 1), ((12, 13, 14, 15), 1)]


def _qblocks(j):
    return [j, 7 - j, j + 8, 15 - j]


def _build(repeat=1, collective=True):
    nc = bacc.Bacc("TRN2", target_bir_lowering=False, debug=False, num_devices=8)

    xtq = nc.dram_tensor("xtq", [D, OWN], bf16, kind="ExternalInput").ap()
    xtv = nc.dram_tensor("xtv", [D, 512], bf16, kind="ExternalInput").ap()
    wqt = nc.dram_tensor("wqt", [D, H * HD], bf16, kind="ExternalInput").ap()
    wkvt = nc.dram_tensor("wkvt", [D, 2 * KV * HD], bf16,
                          kind="ExternalInput").ap()
    costv8 = nc.dram_tensor("costv8", [512, 512], f32, kind="ExternalInput").ap()
    sintv8 = nc.dram_tensor("sintv8", [512, 512], f32, kind="ExternalInput").ap()
    ident = nc.dram_tensor("ident", [P, P], f32, kind="ExternalInput").ap()
    wot = nc.dram_tensor("wot", [H * HD, D], bf16, kind="ExternalInput").ap()
    costq = nc.dram_tensor("costq", [P, OWN], f32, kind="ExternalInput").ap()
    sintq = nc.dram_tensor("sintq", [P, OWN], f32, kind="ExternalInput").ap()
    masku = nc.dram_tensor("masku", [NB, P, P], bf16, kind="ExternalInput").ap()
    negi = nc.dram_tensor("negi", [P, P], bf16, kind="ExternalInput").ap()
    onesr = nc.dram_tensor("onesr", [1, HD], f32, kind="ExternalInput").ap()
    y = nc.dram_tensor("y", [OWN, D], f32, kind="ExternalOutput").ap()

    def rope_full(dst, src, cosr, sinr, tmp_pool):
        """dst = bf16(src*cos + swap32(src)*sin_signed) over 128 rows.

        cosr rows r = cos[r%64]; sinr rows are sign-folded (-sin for
        (r%64)<32, +sin otherwise). The 32-row half-swaps run on the scalar
        engine (idle during projections); the wide elementwise ops run on
        DVE at full lane utilization."""
        xr = tmp_pool.tile([P, 512], f32, tag="xrot", bufs=4, name="xr")
        for po in (0, 64):
            nc.scalar.copy(xr[po:po + 32, :], src[po + 32:po + 64, :])
            nc.scalar.copy(xr[po + 32:po + 64, :], src[po:po + 32, :])
        u = tmp_pool.tile([P, 512], f32, tag="ropeu", bufs=4, name="u")
        v = tmp_pool.tile([P, 512], f32, tag="ropev", bufs=4, name="v")
        nc.vector.tensor_mul(u[:], src[:], cosr)
        nc.vector.tensor_mul(v[:], xr[:], sinr)
        nc.vector.tensor_add(dst[:], u[:], v[:])

    with tile.TileContext(nc) as tc:
        _dpool_cm = tc.tile_pool(name="dram", bufs=1, space="DRAM")
        dpool = _dpool_cm.__enter__()
        _pers_cm = tc.tile_pool(name="pers", bufs=1)
        pers = _pers_cm.__enter__()

        agink = dpool.tile([AG_K], bf16, tag="agink", name="agink")
        aginv = dpool.tile([AG_V], bf16, tag="aginv", name="aginv")
        agoutk = dpool.tile([4, AG_K], bf16, tag="agoutk", name="agoutk")
        agoutv = dpool.tile([4, AG_V], bf16, tag="agoutv", name="agoutv")

        # persistent across stages
        cosq_t = pers.tile([P, OWN], f32, tag="cosq_t", name="cosq_t")
        sinq_t = pers.tile([P, OWN], f32, tag="sinq_t", name="sinq_t")
        ones_t = pers.tile([1, HD], f32r, tag="ones_t", name="ones_t")
        mask_t = pers.tile([P, NB * P], bf16, tag="mask_t", name="mask_t")
        negi_t = pers.tile([P, P], bf16, tag="negi_t", name="negi_t")
        qtr = [pers.tile([P, OWN], bf16, tag=f"qtr{i}", name=f"qtr{i}")
               for i in range(16)]
        # ktr_all[p, db*2048 + t] = k^T[128*db + p, t]
        ktr_all = pers.tile([P, 4 * T], bf16, tag="ktr_all", name="ktr_all")
        vaug = pers.tile([P, KV * VROW], bf16, tag="vaug", name="vaug")

        def emit_body():
         # ================= stages A-D: projections + AllGather ============
         with tc.tile_pool(name="consA", bufs=1) as cA, \
             tc.tile_pool(name="wk", bufs=1) as wkp, \
             tc.tile_pool(name="wst", bufs=4) as wst, \
             tc.tile_pool(name="xs", bufs=4) as xsp, \
             tc.tile_pool(name="tmp", bufs=4) as tmpp, \
             tc.tile_pool(name="cpy", bufs=3) as cpyp, \
             tc.tile_pool(name="pproj", bufs=1, space="PSUM") as pproj:

            # ---- stages A+B: token-sharded K and V projections ----
            costd = cA.tile([P, 4 * 512], f32, tag="costd", name="costd")
            sintd = cA.tile([P, 4 * 512], f32, tag="sintd", name="sintd")
            idtA = cA.tile([P, P], bf16, tag="idtA", name="idtA")
            idf = cA.tile([P, P], f32, tag="idf", name="idf")
            krT_f = cA.tile([P, 4 * 512], bf16, tag="krT_f", name="krT_f")

            psk = [pproj.tile([P, 512], f32, tag=f"k{i}", bufs=1,
                              name=f"psk{i}") for i in range(4)]
            psv = [pproj.tile([P, 512], f32, tag=f"v{i}", bufs=1,
                              name=f"psv{i}") for i in range(4)]
            xtq_sb = [wkp.tile([P, OWN], bf16, tag=f"xq{cb}", name=f"xq{cb}")
                      for cb in range(16)]
            wqf = [wkp.tile([P, H * HD], bf16, tag=f"wqf{cb}",
                            name=f"wqf{cb}") for cb in range(16)]
            for cb in range(16):
                wkv_t = wst.tile([P, 2 * KV * HD], bf16, tag="wkv",
                                 name="wkv")
                nc.sync.dma_start(wkv_t[:], wkvt[P * cb:P * cb + P, :])
                wk_t = wkv_t[:, 0:KV * HD]
                wv_t = wkv_t[:, KV * HD:2 * KV * HD]
                xv_t = xsp.tile([P, 512], bf16, tag="xtv", name="xtv")
                nc.sync.dma_start(
                    xv_t[:], xtv[P * cb:P * cb + P, :])
                # prefetch streams ride SWDGE (Pool is idle here) so the
                # hot KV loads keep HWDGE to themselves
                nc.gpsimd.dma_start(xtq_sb[cb][:], xtq[P * cb:P * cb + P, :])
                nc.sync.dma_start(wqf[cb][:], wqt[P * cb:P * cb + P, :])
                if cb < 4:
                    nc.gpsimd.dma_start(costd[:, 512 * cb:512 * cb + 512],
                                        costv8[P * cb:P * cb + P, :])
                    nc.gpsimd.dma_start(sintd[:, 512 * cb:512 * cb + 512],
                                        sintv8[P * cb:P * cb + P, :])
                elif cb == 4:
                    nc.gpsimd.dma_start(idf[:], ident[:])
                    nc.scalar.copy(idtA[:], idf[:])
                    nc.gpsimd.dma_start(negi_t[:], negi[:])
                    nc.gpsimd.dma_start(ones_t[:], onesr[:].bitcast(f32r))
                elif cb == 5:
                    nc.gpsimd.dma_start(cosq_t[:], costq[:])
                    nc.gpsimd.dma_start(sinq_t[:], sintq[:])
                elif cb == 6:
                    nc.gpsimd.dma_start(
                        mask_t.rearrange("p (b c) -> p b c", b=NB),
                        masku.rearrange("b p c -> p b c"))
                for vb in range(4):
                    nc.tensor.matmul(psk[vb][:],
                                     lhsT=xv_t[:, P * vb:P * vb + P],
                                     rhs=wk_t,
                                     start=(cb == 0), stop=(cb == 15))
                    nc.tensor.matmul(psv[vb][:],
                                     lhsT=xv_t[:, P * vb:P * vb + P],
                                     rhs=wv_t,
                                     start=(cb == 0), stop=(cb == 15))
            # V shard assembled in vaug layout locally (strided ACT copies +
            # ones columns), then one DMA out
            vs_big = cA.tile([P, VSH], bf16, tag="vs_big", name="vs_big")
            vsv = vs_big.rearrange("p (b k c) -> p b k c", k=KV, c=HD + 1)
            nc.vector.memset(vsv[:, :, :, HD], 1.0)
            for vb in range(4):
                nc.scalar.copy(
                    vsv[:, vb, :, 0:HD],
                    psv[vb].rearrange("p (k c) -> p k c", k=KV))
            nc.gpsimd.dma_start(
                aginv.rearrange("(p c) -> p c", p=P), vs_big[:])
            # K rope in (t, d) layout; transpose to (d, t); write shard
            for vb in range(4):
                cs = costd[:, 512 * vb:512 * vb + 512]
                sn = sintd[:, 512 * vb:512 * vb + 512]
                u = tmpp.tile([P, 512], f32, tag="ropeu", bufs=4, name="u")
                nc.vector.tensor_mul(u[:], psk[vb][:], cs)
                vv = tmpp.tile([P, 512], f32, tag="ropev", bufs=4, name="vv")
                pr = psk[vb].rearrange("p (h s w) -> p h s w", s=2, w=32)
                vr = vv.rearrange("p (h s w) -> p h s w", s=2, w=32)
                sr = sn.rearrange("p (h s w) -> p h s w", s=2, w=32)
                nc.vector.tensor_mul(vr[:, :, 0, :], pr[:, :, 1, :],
                                     sr[:, :, 0, :])
                nc.vector.tensor_mul(vr[:, :, 1, :], pr[:, :, 0, :],
                                     sr[:, :, 1, :])
                kr = cpyp.tile([P, 512], bf16, tag="kr", name="kr")
                nc.vector.tensor_add(kr[:], u[:], vv[:])
                for db in range(4):
                    ptr = pproj.tile([P, P], bf16, tag=f"v{db}", bufs=1,
                                     name="ptrA")
                    nc.tensor.transpose(ptr[:], kr[:, P * db:P * db + P],
                                        idtA[:])
                    dst = krT_f[:, 512 * db + P * vb:512 * db + P * vb + P]
                    if db % 2 == 0:
                        nc.vector.tensor_copy(dst, ptr[:])
                    else:
                        nc.scalar.copy(dst, ptr[:])
            nc.gpsimd.dma_start(
                agink.rearrange("(p c) -> p c", p=P), krT_f[:])

            # ---- stage C: split AllGathers (V first — ready earlier) ----
            if collective:
                nc.gpsimd.collective_compute(
                    "AllGather",
                    mybir.AluOpType.bypass,
                    replica_groups=[[0, 1, 2, 3], [4, 5, 6, 7]],
                    ins=[aginv.opt()],
                    outs=[agoutv.opt()],
                )
                nc.gpsimd.collective_compute(
                    "AllGather",
                    mybir.AluOpType.bypass,
                    replica_groups=[[0, 1, 2, 3], [4, 5, 6, 7]],
                    ins=[agink.opt()],
                    outs=[agoutk.opt()],
                )
            else:
                for g in range(4):
                    nc.gpsimd.dma_start(
                        agoutv[g].rearrange("(a b) -> a b", b=8320),
                        aginv.rearrange("(a b) -> a b", b=8320))
                for g in range(4):
                    nc.gpsimd.dma_start(
                        agoutk[g].rearrange("(a b) -> a b", b=8192),
                        agink.rearrange("(a b) -> a b", b=8192))

            # ---- unpack: both in-gather layouts need 4 DMAs each ----
            for g in range(4):
                ksrc = agoutk[g, 0:AG_K].rearrange(
                    "(p d t) -> p d t", p=P, t=512)
                kdst = ktr_all.rearrange("p (d t) -> p d t", d=4)[
                    :, :, 512 * g:512 * g + 512]
                nc.gpsimd.dma_start(kdst, ksrc)
            for g in range(4):
                nc.gpsimd.dma_start(
                    vaug[:, VSH * g:VSH * g + VSH],
                    agoutv[g, 0:AG_V].rearrange("(p c) -> p c", p=P))

            # ---- stage D: Q projection (owned tokens) + rope ----
            for qg in range(4):
                tg = "k" if qg % 2 == 0 else "v"
                psq = [pproj.tile([P, 512], f32, tag=f"{tg}{i}", bufs=1,
                                  name=f"psq{i}") for i in range(4)]
                for cb in range(16):
                    wq_t = wqf[cb][:, 512 * qg:512 * qg + 512]
                    for qi in range(4):
                        nc.tensor.matmul(
                            psq[qi][:], lhsT=wq_t[:, P * qi:P * qi + P],
                            rhs=xtq_sb[cb], start=(cb == 0), stop=(cb == 15))
                for qi in range(4):
                    rope_full(qtr[4 * qg + qi], psq[qi], cosq_t[:],
                              sinq_t[:], tmpp)

         # ================= stages E-G: attention + fused o_proj ===========
         with tc.tile_pool(name="psS2", bufs=1, space="PSUM") as psS2, \
             tc.tile_pool(name="psS1", bufs=1, space="PSUM") as psS1, \
             tc.tile_pool(name="psO", bufs=1, space="PSUM") as psO, \
             tc.tile_pool(name="ptp", bufs=4) as ptp, \
             tc.tile_pool(name="nrm", bufs=3) as nrm, \
             tc.tile_pool(name="wos", bufs=1) as wos, \
             tc.tile_pool(name="otrg", bufs=1) as otrp, \
             tc.tile_pool(name="ysbp", bufs=1) as ysbp, \
             tc.tile_pool(name="yc", bufs=2) as ycp:

            otr = [otrp.tile([P, OWN], bf16, tag=f"otr{i % 8}",
                             name=f"otr{i}") for i in range(16)]
            ysb = {}
            for eb in range(4):
                for tb in range(4):
                    ysb[(eb, tb)] = ysbp.tile(
                        [P, 512], bf16, tag=f"ysb{eb}_{tb}",
                        name=f"ysb{eb}_{tb}")

            # wo chunk prefetch: o_proj contraction chunk ci covers
            # ab in CHUNKS[ci]; prefetched two chunks ahead.
            CHUNKS = [(0, 4), (4, 8), (8, 12), (12, 15), (15, 16)]
            wo_ch = {}

            def prefetch_chunk(ci):
                for ab in range(*CHUNKS[ci]):
                    for eb in range(4):
                        wt = wos.tile([P, 512], bf16, tag=f"wo{ab % 8}_{eb}",
                                      name=f"wo{ab}_{eb}", bufs=1)
                        nc.sync.dma_start(
                            wt[:], wot[P * ab:P * ab + P,
                                       512 * eb:512 * eb + 512])
                        wo_ch[(eb, ab)] = wt

            prefetch_chunk(0)
            prefetch_chunk(1)

            # o_proj work queue: (q, eb, tb) quanta emitted between attention
            # groups to fill the PE; the PSUM slot tag is picked at pop time
            # (the score tags join the rotation once attention has drained).
            oproj_work = []

            def enqueue_chunk(ci):
                for eb in range(4):
                    for tb in range(4):
                        oproj_work.append((ci, eb, tb))

            def emit_oproj(ci, eb, tb, tag):
                lo, hi = CHUNKS[ci]
                pool = psO if tag == "oaug" else (
                    psS1 if tag == "st1" else psS2)
                psg = pool.tile([P, 512], f32, tag=tag, bufs=2, name="psg")
                for ab in range(lo, hi):
                    nc.tensor.matmul(
                        psg[:], lhsT=otr[ab][:, P * tb:P * tb + P],
                        rhs=wo_ch[(eb, ab)],
                        start=(ab == lo), stop=(ab == hi - 1))
                t = ysb[(eb, tb)]
                if ci == 0:
                    nc.vector.tensor_copy(t[:], psg[:])
                elif ci < len(CHUNKS) - 1:
                    nc.vector.tensor_add(t[:], t[:], psg[:])
                else:
                    yt = ycp.tile([P, 512], f32, tag="yt", bufs=6,
                                  name="yt")
                    nc.vector.tensor_add(yt[:], t[:], psg[:])
                    eng = nc.sync if (eb + tb) % 2 == 0 else nc.gpsimd
                    eng.dma_start(
                        y[P * tb:P * tb + P, 512 * eb:512 * eb + 512], yt[:])

            def pop_oproj(k, tags=("oaug",)):
                for i in range(min(k, len(oproj_work))):
                    ci, eb, tb = oproj_work.pop(0)
                    emit_oproj(ci, eb, tb, tags[i % len(tags)])

            def emit_group(st, pt, kbs, kind, w, v4, m, side):
                qs = P * (kbs[0] // 4)
                n = OWN - qs
                kpo = HD * side
                kt = ktr_all[kpo:kpo + HD,
                             2048 * v4:2048 * v4 + 2048]
                rhs = qtr[4 * v4 + m][kpo:kpo + HD, qs:OWN]
                for i, kb in enumerate(kbs):
                    nc.tensor.matmul(
                        st[:, w * i:w * i + n],
                        lhsT=kt[:, P * kb:P * kb + P],
                        rhs=rhs,
                        start=(kind == 2 or i == 0), stop=False,
                        skip_group_check=True)
                if len(kbs) == 4:
                    nc.tensor.matmul(
                        st[:, 0:512], lhsT=negi_t[:],
                        rhs=mask_t[:, P * kbs[0]:P * kbs[0] + 512],
                        start=False, stop=True, skip_group_check=True)
                else:
                    for i, kb in enumerate(kbs):
                        nc.tensor.matmul(
                            st[:, w * i:w * i + P], lhsT=negi_t[:],
                            rhs=mask_t[:, P * kb:P * kb + P],
                            start=False, stop=(i == len(kbs) - 1),
                            skip_group_check=True)
                # one exp per group (strided across banks for kind 2)
                if kind == 2 and n < 512:
                    stv = st.rearrange("p (s c) -> p s c", s=2)
                    ptv = pt.rearrange("p (s c) -> p s c", s=2)
                    nc.scalar.activation(ptv[:, :, 0:n], stv[:, :, 0:n],
                                         Exp, scale=0.125)
                else:
                    m = w * (len(kbs) - 1) + n
                    nc.scalar.activation(pt[:, 0:m], st[:, 0:m], Exp,
                                         scale=0.125)

            for uu in range(16):
                v4 = uu // 4
                # head pair interleaved at group granularity: tile uu rows
                # 0:64 = head a (kv 2*v4), rows 64:128 = head a+4 (2*v4+1)
                m_u = uu % 4
                oaug = [psO.tile([P, OWN], f32, tag="oaug", bufs=2,
                                 name=f"oaug{s_}") for s_ in range(2)]
                pend = {0: [], 1: []}  # per side, AV lag 2
                for kbs, kind in GROUPS:
                    for side in range(2):
                        kh = 2 * v4 + side
                        if kind == 2:
                            st = psS2.tile([P, 1024], f32, tag="st2",
                                           bufs=2, name="st2")
                            w = 512
                        else:
                            st = psS1.tile([P, 512], f32, tag="st1",
                                           bufs=2, name="st1")
                            w = 512 // len(kbs)
                        pt = ptp.tile([P, 1024], bf16, tag="pt", bufs=10,
                                      name="pt")
                        emit_group(st, pt, kbs, kind, w, v4, m_u, side)
                        pend[side].append((pt, kbs, w))
                        if len(pend[side]) > 2:
                            _emit_avs(nc, pend[side].pop(0), vaug,
                                      oaug[side], kh)
                for side in range(2):
                    for pe_ in pend[side]:
                        _emit_avs(nc, pe_, vaug, oaug[side], 2 * v4 + side)
                for side in range(2):
                    # normalization: rec = 1/sums; Pool broadcasts to 64 rows
                    rec = nrm.tile([1, OWN], f32, tag="rec", name="rec")
                    nc.vector.reciprocal(rec[:], oaug[side][HD:HD + 1, :])
                    pbs = nrm.tile([HD, OWN], f32, tag="pbs", bufs=3,
                                   name="pbs")
                    nc.gpsimd.partition_broadcast(pbs[:], rec[:])
                    nc.vector.tensor_mul(
                        otr[uu][HD * side:HD * side + HD, :],
                        oaug[side][0:HD, :], pbs[:])
                pop_oproj(4, tags=("st1",))
                ci = {3: 0, 7: 1, 11: 2, 13: 3, 15: 4}.get(uu)
                if ci is not None:
                    enqueue_chunk(ci)
                    if ci + 2 < len(CHUNKS):
                        prefetch_chunk(ci + 2)
            pop_oproj(len(oproj_work), tags=("oaug", "st1", "st2"))

        for _rep in range(repeat):
            emit_body()

        _pers_cm.__exit__(None, None, None)
        _dpool_cm.__exit__(None, None, None)

    nc.compile()
    return nc


def _emit_avs(nc, pend, vaug, oaug, kh):
    pt, kbs, w = pend
    qs = P * (kbs[0] // 4)
    n = OWN - qs
    for i, kb in enumerate(kbs):
        base = (HD + 1) * (KV * kb + kh)
        nc.tensor.matmul(
            oaug[0:HD + 1, qs:OWN],
            lhsT=vaug[:, base:base + HD + 1],
            rhs=pt[:, w * i:w * i + n],
            start=(kb == 0), stop=(kb == NB - 1))


_NC = None


def _get_nc():
    global _NC
    if _NC is None:
        _NC = _build()
    return _NC


def _head_perm():
    """Pair each even-kv head with its odd-kv partner (+4) in one 128-dim
    block, so q partition parity matches the kv head parity in kT tiles."""
    order = []
    for u in range(16):
        a = 8 * (u // 4) + u % 4
        for h in (a, a + 4):
            order.extend(range(HD * h, HD * h + HD))
    return np.asarray(order)


def _in_maps(x, cos, sin, Wq, Wk, Wv, Wo):
    xT = np.ascontiguousarray(np.transpose(np.asarray(x, np.float32), (0, 2, 1)))
    perm = _head_perm()
    import ml_dtypes as _mld
    WqT = np.ascontiguousarray(
        np.asarray(Wq, np.float32).T[:, perm].astype(_mld.bfloat16))
    WkvT = np.ascontiguousarray(np.concatenate(
        [np.asarray(Wk, np.float32).T, np.asarray(Wv, np.float32).T],
        axis=1).astype(_mld.bfloat16))
    WoT = np.ascontiguousarray(
        np.asarray(Wo, np.float32).T[perm, :].astype(_mld.bfloat16))
    cosT = np.asarray(cos, np.float32).T        # (64, T)
    sinT = np.asarray(sin, np.float32).T
    # 128-row rope tables: row r uses hd-dim r%64; sin rows sign-folded
    # (-sin for (r%64)<32) so rope is x*cos2 + swap32(x)*sin2 on 128 rows.
    sgn = np.where(np.arange(HD) < HD // 2, -1.0, 1.0).astype(np.float32)
    sgnc = sgn  # same sign-fold along the free axis for (t, d) layout
    cos2 = np.ascontiguousarray(np.tile(cosT, (2, 1)))              # (128, T)
    sin2 = np.ascontiguousarray(np.tile(sinT * sgn[:, None], (2, 1)))
    ones = np.ones((1, HD), np.float32)
    maps = []
    for c in range(8):
        b, j = c // 4, c % 4
        qb = _qblocks(j)
        cols = np.concatenate([np.arange(P * g, P * g + P) for g in qb])
        mask = np.empty((NB, P, P), _mld.bfloat16)
        ki = np.arange(P)[:, None]
        qi = np.arange(P)[None, :]
        for kb in range(NB):
            qg = qb[kb // 4]
            mask[kb] = np.where(P * kb + ki <= P * qg + qi, 0.0, 1.0)
        negi_np = (np.eye(P, dtype=np.float32) *
                   np.float32(-2.0 ** 30)).astype(_mld.bfloat16)
        maps.append({
            "xtq": np.ascontiguousarray(
                xT[b][:, cols].astype(_mld.bfloat16)),
            "xtv": np.ascontiguousarray(
                xT[b][:, 512 * j:512 * j + 512].astype(_mld.bfloat16)),
            "wqt": WqT,
            "wkvt": WkvT,
            "wot": WoT,
            "costq": np.ascontiguousarray(cos2[:, cols]),
            "sintq": np.ascontiguousarray(sin2[:, cols]),
            "costv8": np.ascontiguousarray(
                np.tile(cosT.T[512 * j:512 * j + 512, :], (1, KV))),
            "sintv8": np.ascontiguousarray(
                np.tile(sinT.T[512 * j:512 * j + 512, :] * sgnc[None, :],
                        (1, KV))),
            "ident": np.eye(P, dtype=np.float32),
            "masku": mask,
            "negi": negi_np,
            "onesr": ones,
        })
    return maps


def kernel(x, cos, sin, Wq, Wk, Wv, Wo):
    nc = _get_nc()
    maps = _in_maps(x, cos, sin, Wq, Wk, Wv, Wo)
    res = run_bass_kernel_spmd(nc, maps, list(range(8)))
    out = np.empty((B, T, D), np.float32)
    for c in range(8):
        b, j = c // 4, c % 4
        yc = res.results[c]["y"]
        for s, qg in enumerate(_qblocks(j)):
            out[b, P * qg:P * qg + P, :] = yc[P * s:P * s + P, :]
    return out


# revision 57
# speedup vs baseline: 1149.1433x; 1.0019x over previous
"""Bass/Trainium2 SPMD kernel for GQA causal attention with RoPE.

Sharding (8 cores): core c = 4*b + j (b = batch, j = 0..3 shard in batch).
  - Q / attention / o_proj: token-sharded; core j owns q-token 128-blocks
    {j, 7-j, j+8, 15-j} (balanced causal work, uniform SPMD program with
    fixed per-slot key extents [512, 1024, 1536, 2048] and data-driven masks).
  - K and V: token-sharded (core j computes tokens [512j, 512j+512), all
    dims; K roped in (t, d) layout). Split AllGathers (K first, then V, both
    bf16) assemble full K and V; K is PE-transposed on chip to (d, t) tiles.
  - Attention in s^T = (kpos, q) layout: scores = k^T.T @ q^T, exp on ACT
    (scale folds 1/sqrt(hd)), AV with ones-augmented V gives softmax sums,
    division applied after AV (commutes with the linear AV/o_proj steps).

v2 structure (cost-model driven):
  - bf16 q/k/v/p operand path: every attention matmul runs 1 cycle/row at
    any free size (fp32r pays 4x below 256), and the K/V AllGather traffic
    halves.
  - exp batching: scores for 2 kb per PSUM group tile ([128,1024] = 2 banks
    for kb 0-7; bank-packed [128,512] tiles for kb 8-15 exploiting the
    PSUM zero-region overwrite semantics) -> 7 exps/head instead of 16.
  - o_proj runs interleaved with attention: after each quad of otr blocks
    completes, its 64 matmuls + DVE adds into bf16 ysb partials are spread
    across the following heads' PE stream (PSUM slots shared with oaug/pb
    via a common pool tag).
  - vaug assembled by 4D-AP DMAs straight from the gathered V (no ACT/DVE
    copies); division output written directly into otr SBUF tiles (no DRAM
    round-trip through obuf).
  - DMA issue split: SP carries the load streams, GPSIMD/SWDGE carries the
    collective-adjacent stores + unpack (25ns issue vs 565ns, and no HWDGE
    contention).
"""
import numpy as np

import concourse.bass as bass
import concourse.tile as tile
from concourse import bacc, mybir
from concourse.bass_utils import run_bass_kernel_spmd

B, T, D = 2, 2048, 2048
H, KV, HD = 32, 8, 64
P = 128
NB = T // P          # 16 token blocks of 128
OWN = 4 * P          # 512 owned q tokens per core
f32 = mybir.dt.float32
f32r = mybir.dt.float32r
bf16 = mybir.dt.bfloat16
Exp = mybir.ActivationFunctionType.Exp
fp8 = mybir.dt.float8e4
DR = mybir.MatmulPerfMode.DoubleRow

AG_K = P * T           # 262144 fp8 elems of folded kT shard
VSH = 4 * KV * (HD + 1)  # 2080 v-shard columns (vaug layout, ones included)
AG_V = P * VSH         # 266240 elems of v shard
VROW = NB * (HD + 1)   # vaug length per kv head in (kb, kh) indexing

# attention group plan: (kbs, kind); kind 2 = [P,1024] two-bank tile with
# one kb per bank, kind 1 = [P,512] single bank packed with 2 or 4 kbs
GROUPS = [((0, 1), 2), ((2, 3), 2), ((4, 5), 2), ((6, 7), 2),
          ((8, 9), 1), ((10, 11), 1), ((12, 13, 14, 15), 1)]


def _qblocks(j):
    return [j, 7 - j, j + 8, 15 - j]


def _build(repeat=1, collective=True):
    nc = bacc.Bacc("TRN2", target_bir_lowering=False, debug=False, num_devices=8)

    xtq = nc.dram_tensor("xtq", [D, OWN], bf16, kind="ExternalInput").ap()
    xtv = nc.dram_tensor("xtv", [D, 512], bf16, kind="ExternalInput").ap()
    wqt = nc.dram_tensor("wqt", [D, H * HD], bf16, kind="ExternalInput").ap()
    wkvt = nc.dram_tensor("wkvt", [D, 2 * KV * HD], bf16,
                          kind="ExternalInput").ap()
    costv8 = nc.dram_tensor("costv8", [512, 512], f32, kind="ExternalInput").ap()
    sintv8 = nc.dram_tensor("sintv8", [512, 512], f32, kind="ExternalInput").ap()
    ident = nc.dram_tensor("ident", [P, P], f32, kind="ExternalInput").ap()
    wot = nc.dram_tensor("wot", [H * HD, D], bf16, kind="ExternalInput").ap()
    costq = nc.dram_tensor("costq", [P, OWN], f32, kind="ExternalInput").ap()
    sintq = nc.dram_tensor("sintq", [P, OWN], f32, kind="ExternalInput").ap()
    masku = nc.dram_tensor("masku", [NB, P, P], bf16, kind="ExternalInput").ap()
    negi = nc.dram_tensor("negi", [P, P], bf16, kind="ExternalInput").ap()
    onesr = nc.dram_tensor("onesr", [1, HD], f32, kind="ExternalInput").ap()
    y = nc.dram_tensor("y", [OWN, D], f32, kind="ExternalOutput").ap()

    def rope_full(dst, src, cosr, sinr, tmp_pool):
        """dst = bf16(src*cos + swap32(src)*sin_signed) over 128 rows.

        cosr rows r = cos[r%64]; sinr rows are sign-folded (-sin for
        (r%64)<32, +sin otherwise). The 32-row half-swaps run on the scalar
        engine (idle during projections); the wide elementwise ops run on
        DVE at full lane utilization."""
        xr = tmp_pool.tile([P, 512], f32, tag="xrot", bufs=4, name="xr")
        for po in (0, 64):
            nc.scalar.copy(xr[po:po + 32, :], src[po + 32:po + 64, :])
            nc.scalar.copy(xr[po + 32:po + 64, :], src[po:po + 32, :])
        u = tmp_pool.tile([P, 512], f32, tag="ropeu", bufs=4, name="u")
        v = tmp_pool.tile([P, 512], f32, tag="ropev", bufs=4, name="v")
        nc.vector.tensor_mul(u[:], src[:], cosr)
        nc.vector.tensor_mul(v[:], xr[:], sinr)
        nc.vector.tensor_add(dst[:], u[:], v[:])

    with tile.TileContext(nc) as tc:
        _dpool_cm = tc.tile_pool(name="dram", bufs=1, space="DRAM")
        dpool = _dpool_cm.__enter__()
        _pers_cm = tc.tile_pool(name="pers", bufs=1)
        pers = _pers_cm.__enter__()

        agink = dpool.tile([AG_K], bf16, tag="agink", name="agink")
        aginv = dpool.tile([AG_V], bf16, tag="aginv", name="aginv")
        agoutk = dpool.tile([4, AG_K], bf16, tag="agoutk", name="agoutk")
        agoutv = dpool.tile([4, AG_V], bf16, tag="agoutv", name="agoutv")

        # persistent across stages
        cosq_t = pers.tile([P, OWN], f32, tag="cosq_t", name="cosq_t")
        sinq_t = pers.tile([P, OWN], f32, tag="sinq_t", name="sinq_t")
        ones_t = pers.tile([1, HD], f32r, tag="ones_t", name="ones_t")
        mask_t = pers.tile([P, NB * P], bf16, tag="mask_t", name="mask_t")
        negi_t = pers.tile([P, P], bf16, tag="negi_t", name="negi_t")
        qtr = [pers.tile([P, OWN], bf16, tag=f"qtr{i}", name=f"qtr{i}")
               for i in range(16)]
        # ktr_all[p, db*2048 + t] = k^T[128*db + p, t]
        ktr_all = pers.tile([P, 4 * T], bf16, tag="ktr_all", name="ktr_all")
        vaug = pers.tile([P, KV * VROW], bf16, tag="vaug", name="vaug")

        def emit_body():
         # ================= stages A-D: projections + AllGather ============
         with tc.tile_pool(name="consA", bufs=1) as cA, \
             tc.tile_pool(name="wk", bufs=1) as wkp, \
             tc.tile_pool(name="wst", bufs=4) as wst, \
             tc.tile_pool(name="xs", bufs=4) as xsp, \
             tc.tile_pool(name="tmp", bufs=4) as tmpp, \
             tc.tile_pool(name="cpy", bufs=3) as cpyp, \
             tc.tile_pool(name="pproj", bufs=1, space="PSUM") as pproj:

            # ---- stages A+B: token-sharded K and V projections ----
            costd = cA.tile([P, 4 * 512], f32, tag="costd", name="costd")
            sintd = cA.tile([P, 4 * 512], f32, tag="sintd", name="sintd")
            idtA = cA.tile([P, P], bf16, tag="idtA", name="idtA")
            idf = cA.tile([P, P], f32, tag="idf", name="idf")
            krT_f = cA.tile([P, 4 * 512], bf16, tag="krT_f", name="krT_f")

            psk = [pproj.tile([P, 512], f32, tag=f"k{i}", bufs=1,
                              name=f"psk{i}") for i in range(4)]
            psv = [pproj.tile([P, 512], f32, tag=f"v{i}", bufs=1,
                              name=f"psv{i}") for i in range(4)]
            xtq_sb = [wkp.tile([P, OWN], bf16, tag=f"xq{cb}", name=f"xq{cb}")
                      for cb in range(16)]
            wqf = [wkp.tile([P, H * HD], bf16, tag=f"wqf{cb}",
                            name=f"wqf{cb}") for cb in range(16)]
            for cb in range(16):
                wkv_t = wst.tile([P, 2 * KV * HD], bf16, tag="wkv",
                                 name="wkv")
                nc.sync.dma_start(wkv_t[:], wkvt[P * cb:P * cb + P, :])
                wk_t = wkv_t[:, 0:KV * HD]
                wv_t = wkv_t[:, KV * HD:2 * KV * HD]
                xv_t = xsp.tile([P, 512], bf16, tag="xtv", name="xtv")
                nc.sync.dma_start(
                    xv_t[:], xtv[P * cb:P * cb + P, :])
                # prefetch streams ride SWDGE (Pool is idle here) so the
                # hot KV loads keep HWDGE to themselves
                nc.gpsimd.dma_start(xtq_sb[cb][:], xtq[P * cb:P * cb + P, :])
                nc.sync.dma_start(wqf[cb][:], wqt[P * cb:P * cb + P, :])
                if cb < 4:
                    nc.gpsimd.dma_start(costd[:, 512 * cb:512 * cb + 512],
                                        costv8[P * cb:P * cb + P, :])
                    nc.gpsimd.dma_start(sintd[:, 512 * cb:512 * cb + 512],
                                        sintv8[P * cb:P * cb + P, :])
                elif cb == 4:
                    nc.gpsimd.dma_start(idf[:], ident[:])
                    nc.scalar.copy(idtA[:], idf[:])
                    nc.gpsimd.dma_start(negi_t[:], negi[:])
                    nc.gpsimd.dma_start(ones_t[:], onesr[:].bitcast(f32r))
                elif cb == 5:
                    nc.gpsimd.dma_start(cosq_t[:], costq[:])
                    nc.gpsimd.dma_start(sinq_t[:], sintq[:])
                elif cb == 6:
                    nc.gpsimd.dma_start(
                        mask_t.rearrange("p (b c) -> p b c", b=NB),
                        masku.rearrange("b p c -> p b c"))
                for vb in range(4):
                    nc.tensor.matmul(psk[vb][:],
                                     lhsT=xv_t[:, P * vb:P * vb + P],
                                     rhs=wk_t,
                                     start=(cb == 0), stop=(cb == 15))
                    nc.tensor.matmul(psv[vb][:],
                                     lhsT=xv_t[:, P * vb:P * vb + P],
                                     rhs=wv_t,
                                     start=(cb == 0), stop=(cb == 15))
            # V shard assembled in vaug layout locally (strided ACT copies +
            # ones columns), then one DMA out
            vs_big = cA.tile([P, VSH], bf16, tag="vs_big", name="vs_big")
            vsv = vs_big.rearrange("p (b k c) -> p b k c", k=KV, c=HD + 1)
            nc.vector.memset(vsv[:, :, :, HD], 1.0)
            for vb in range(4):
                nc.scalar.copy(
                    vsv[:, vb, :, 0:HD],
                    psv[vb].rearrange("p (k c) -> p k c", k=KV))
            nc.gpsimd.dma_start(
                aginv.rearrange("(p c) -> p c", p=P), vs_big[:])
            # K rope in (t, d) layout; transpose to (d, t); write shard
            for vb in range(4):
                cs = costd[:, 512 * vb:512 * vb + 512]
                sn = sintd[:, 512 * vb:512 * vb + 512]
                u = tmpp.tile([P, 512], f32, tag="ropeu", bufs=4, name="u")
                nc.vector.tensor_mul(u[:], psk[vb][:], cs)
                vv = tmpp.tile([P, 512], f32, tag="ropev", bufs=4, name="vv")
                pr = psk[vb].rearrange("p (h s w) -> p h s w", s=2, w=32)
                vr = vv.rearrange("p (h s w) -> p h s w", s=2, w=32)
                sr = sn.rearrange("p (h s w) -> p h s w", s=2, w=32)
                nc.vector.tensor_mul(vr[:, :, 0, :], pr[:, :, 1, :],
                                     sr[:, :, 0, :])
                nc.vector.tensor_mul(vr[:, :, 1, :], pr[:, :, 0, :],
                                     sr[:, :, 1, :])
                kr = cpyp.tile([P, 512], bf16, tag="kr", name="kr")
                nc.vector.tensor_add(kr[:], u[:], vv[:])
                for db in range(4):
                    ptr = pproj.tile([P, P], bf16, tag=f"v{db}", bufs=1,
                                     name="ptrA")
                    nc.tensor.transpose(ptr[:], kr[:, P * db:P * db + P],
                                        idtA[:])
                    dst = krT_f[:, 512 * db + P * vb:512 * db + P * vb + P]
                    if db % 2 == 0:
                        nc.vector.tensor_copy(dst, ptr[:])
                    else:
                        nc.scalar.copy(dst, ptr[:])
            nc.gpsimd.dma_start(
                agink.rearrange("(p c) -> p c", p=P), krT_f[:])

            # ---- stage C: split AllGathers (V first — ready earlier) ----
            if collective:
                nc.gpsimd.collective_compute(
                    "AllGather",
                    mybir.AluOpType.bypass,
                    replica_groups=[[0, 1, 2, 3], [4, 5, 6, 7]],
                    ins=[aginv.opt()],
                    outs=[agoutv.opt()],
                )
                nc.gpsimd.collective_compute(
                    "AllGather",
                    mybir.AluOpType.bypass,
                    replica_groups=[[0, 1, 2, 3], [4, 5, 6, 7]],
                    ins=[agink.opt()],
                    outs=[agoutk.opt()],
                )
            else:
                for g in range(4):
                    nc.gpsimd.dma_start(
                        agoutv[g].rearrange("(a b) -> a b", b=8320),
                        aginv.rearrange("(a b) -> a b", b=8320))
                for g in range(4):
                    nc.gpsimd.dma_start(
                        agoutk[g].rearrange("(a b) -> a b", b=8192),
                        agink.rearrange("(a b) -> a b", b=8192))

            # ---- unpack: both in-gather layouts need 4 DMAs each ----
            for g in range(4):
                ksrc = agoutk[g, 0:AG_K].rearrange(
                    "(p d t) -> p d t", p=P, t=512)
                kdst = ktr_all.rearrange("p (d t) -> p d t", d=4)[
                    :, :, 512 * g:512 * g + 512]
                nc.gpsimd.dma_start(kdst, ksrc)
            for g in range(4):
                nc.gpsimd.dma_start(
                    vaug[:, VSH * g:VSH * g + VSH],
                    agoutv[g, 0:AG_V].rearrange("(p c) -> p c", p=P))

            # ---- stage D: Q projection (owned tokens) + rope ----
            for qg in range(4):
                tg = "k" if qg % 2 == 0 else "v"
                psq = [pproj.tile([P, 512], f32, tag=f"{tg}{i}", bufs=1,
                                  name=f"psq{i}") for i in range(4)]
                for cb in range(16):
                    wq_t = wqf[cb][:, 512 * qg:512 * qg + 512]
                    for qi in range(4):
                        nc.tensor.matmul(
                            psq[qi][:], lhsT=wq_t[:, P * qi:P * qi + P],
                            rhs=xtq_sb[cb], start=(cb == 0), stop=(cb == 15))
                for qi in range(4):
                    rope_full(qtr[4 * qg + qi], psq[qi], cosq_t[:],
                              sinq_t[:], tmpp)

         # ================= stages E-G: attention + fused o_proj ===========
         with tc.tile_pool(name="psS2", bufs=1, space="PSUM") as psS2, \
             tc.tile_pool(name="psS1", bufs=1, space="PSUM") as psS1, \
             tc.tile_pool(name="psO", bufs=1, space="PSUM") as psO, \
             tc.tile_pool(name="ptp", bufs=4) as ptp, \
             tc.tile_pool(name="nrm", bufs=3) as nrm, \
             tc.tile_pool(name="wos", bufs=1) as wos, \
             tc.tile_pool(name="otrg", bufs=1) as otrp, \
             tc.tile_pool(name="ysbp", bufs=1) as ysbp, \
             tc.tile_pool(name="yc", bufs=2) as ycp:

            otr = [otrp.tile([P, OWN], bf16, tag=f"otr{i % 8}",
                             name=f"otr{i}") for i in range(16)]
            ysb = {}
            for eb in range(4):
                for tb in range(4):
                    ysb[(eb, tb)] = ysbp.tile(
                        [P, 512], bf16, tag=f"ysb{eb}_{tb}",
                        name=f"ysb{eb}_{tb}")

            # wo chunk prefetch: o_proj contraction chunk ci covers
            # ab in CHUNKS[ci]; prefetched two chunks ahead.
            CHUNKS = [(0, 4), (4, 8), (8, 12), (12, 14), (14, 16)]
            wo_ch = {}

            def prefetch_chunk(ci):
                for ab in range(*CHUNKS[ci]):
                    for eb in range(4):
                        wt = wos.tile([P, 512], bf16, tag=f"wo{ab % 8}_{eb}",
                                      name=f"wo{ab}_{eb}", bufs=1)
                        nc.sync.dma_start(
                            wt[:], wot[P * ab:P * ab + P,
                                       512 * eb:512 * eb + 512])
                        wo_ch[(eb, ab)] = wt

            prefetch_chunk(0)
            prefetch_chunk(1)

            # o_proj work queue: (q, eb, tb) quanta emitted between attention
            # groups to fill the PE; the PSUM slot tag is picked at pop time
            # (the score tags join the rotation once attention has drained).
            oproj_work = []

            def enqueue_chunk(ci):
                for eb in range(4):
                    for tb in range(4):
                        oproj_work.append((ci, eb, tb))

            def emit_oproj(ci, eb, tb, tag):
                lo, hi = CHUNKS[ci]
                pool = psO if tag == "oaug" else (
                    psS1 if tag == "st1" else psS2)
                psg = pool.tile([P, 512], f32, tag=tag, bufs=2, name="psg")
                for ab in range(lo, hi):
                    nc.tensor.matmul(
                        psg[:], lhsT=otr[ab][:, P * tb:P * tb + P],
                        rhs=wo_ch[(eb, ab)],
                        start=(ab == lo), stop=(ab == hi - 1))
                t = ysb[(eb, tb)]
                if ci == 0:
                    nc.vector.tensor_copy(t[:], psg[:])
                elif ci < len(CHUNKS) - 1:
                    nc.vector.tensor_add(t[:], t[:], psg[:])
                else:
                    yt = ycp.tile([P, 512], f32, tag="yt", bufs=6,
                                  name="yt")
                    nc.vector.tensor_add(yt[:], t[:], psg[:])
                    eng = nc.sync if (eb + tb) % 2 == 0 else nc.gpsimd
                    eng.dma_start(
                        y[P * tb:P * tb + P, 512 * eb:512 * eb + 512], yt[:])

            def pop_oproj(k, tags=("oaug",)):
                for i in range(min(k, len(oproj_work))):
                    ci, eb, tb = oproj_work.pop(0)
                    emit_oproj(ci, eb, tb, tags[i % len(tags)])

            def emit_group(st, pt, kbs, kind, w, v4, m, side):
                qs = P * (kbs[0] // 4)
                n = OWN - qs
                kpo = HD * side
                kt = ktr_all[kpo:kpo + HD,
                             2048 * v4:2048 * v4 + 2048]
                rhs = qtr[4 * v4 + m][kpo:kpo + HD, qs:OWN]
                for i, kb in enumerate(kbs):
                    nc.tensor.matmul(
                        st[:, w * i:w * i + n],
                        lhsT=kt[:, P * kb:P * kb + P],
                        rhs=rhs,
                        start=(kind == 2 or i == 0), stop=False,
                        skip_group_check=True)
                if len(kbs) == 4:
                    nc.tensor.matmul(
                        st[:, 0:512], lhsT=negi_t[:],
                        rhs=mask_t[:, P * kbs[0]:P * kbs[0] + 512],
                        start=False, stop=True, skip_group_check=True)
                else:
                    for i, kb in enumerate(kbs):
                        nc.tensor.matmul(
                            st[:, w * i:w * i + P], lhsT=negi_t[:],
                            rhs=mask_t[:, P * kb:P * kb + P],
                            start=False, stop=(i == len(kbs) - 1),
                            skip_group_check=True)
                # one exp per group (strided across banks for kind 2)
                if kind == 2 and n < 512:
                    stv = st.rearrange("p (s c) -> p s c", s=2)
                    ptv = pt.rearrange("p (s c) -> p s c", s=2)
                    nc.scalar.activation(ptv[:, :, 0:n], stv[:, :, 0:n],
                                         Exp, scale=0.125)
                else:
                    m = w * (len(kbs) - 1) + n
                    nc.scalar.activation(pt[:, 0:m], st[:, 0:m], Exp,
                                         scale=0.125)

            for uu in range(16):
                v4 = uu // 4
                # head pair interleaved at group granularity: tile uu rows
                # 0:64 = head a (kv 2*v4), rows 64:128 = head a+4 (2*v4+1)
                m_u = uu % 4
                oaug = [psO.tile([P, OWN], f32, tag="oaug", bufs=2,
                                 name=f"oaug{s_}") for s_ in range(2)]
                pend = {0: [], 1: []}  # per side, AV lag 2
                for kbs, kind in GROUPS:
                    for side in range(2):
                        kh = 2 * v4 + side
                        if kind == 2:
                            st = psS2.tile([P, 1024], f32, tag="st2",
                                           bufs=2, name="st2")
                            w = 512
                        else:
                            st = psS1.tile([P, 512], f32, tag="st1",
                                           bufs=2, name="st1")
                            w = 512 // len(kbs)
                        pt = ptp.tile([P, 1024], bf16, tag="pt", bufs=10,
                                      name="pt")
                        emit_group(st, pt, kbs, kind, w, v4, m_u, side)
                        pend[side].append((pt, kbs, w))
                        if len(pend[side]) > 2:
                            _emit_avs(nc, pend[side].pop(0), vaug,
                                      oaug[side], kh)
                for side in range(2):
                    for pe_ in pend[side]:
                        _emit_avs(nc, pe_, vaug, oaug[side], 2 * v4 + side)
                for side in range(2):
                    # normalization: rec = 1/sums; Pool broadcasts to 64 rows
                    rec = nrm.tile([1, OWN], f32, tag="rec", name="rec")
                    nc.vector.reciprocal(rec[:], oaug[side][HD:HD + 1, :])
                    pbs = nrm.tile([HD, OWN], f32, tag="pbs", bufs=3,
                                   name="pbs")
                    nc.gpsimd.partition_broadcast(pbs[:], rec[:])
                    nc.vector.tensor_mul(
                        otr[uu][HD * side:HD * side + HD, :],
                        oaug[side][0:HD, :], pbs[:])
                pop_oproj(4, tags=("st1",))
                ci = {3: 0, 7: 1, 11: 2, 13: 3, 15: 4}.get(uu)
                if ci is not None:
                    enqueue_chunk(ci)
                    if ci + 2 < len(CHUNKS):
                        prefetch_chunk(ci + 2)
            pop_oproj(len(oproj_work), tags=("oaug", "st1", "st2"))

        for _rep in range(repeat):
            emit_body()

        _pers_cm.__exit__(None, None, None)
        _dpool_cm.__exit__(None, None, None)

    nc.compile()
    return nc


def _emit_avs(nc, pend, vaug, oaug, kh):
    pt, kbs, w = pend
    qs = P * (kbs[0] // 4)
    n = OWN - qs
    for i, kb in enumerate(kbs):
        base = (HD + 1) * (KV * kb + kh)
        nc.tensor.matmul(
            oaug[0:HD + 1, qs:OWN],
            lhsT=vaug[:, base:base + HD + 1],
            rhs=pt[:, w * i:w * i + n],
            start=(kb == 0), stop=(kb == NB - 1))


_NC = None


def _get_nc():
    global _NC
    if _NC is None:
        _NC = _build()
    return _NC


def _head_perm():
    """Pair each even-kv head with its odd-kv partner (+4) in one 128-dim
    block, so q partition parity matches the kv head parity in kT tiles."""
    order = []
    for u in range(16):
        a = 8 * (u // 4) + u % 4
        for h in (a, a + 4):
            order.extend(range(HD * h, HD * h + HD))
    return np.asarray(order)


def _in_maps(x, cos, sin, Wq, Wk, Wv, Wo):
    xT = np.ascontiguousarray(np.transpose(np.asarray(x, np.float32), (0, 2, 1)))
    perm = _head_perm()
    import ml_dtypes as _mld
    WqT = np.ascontiguousarray(
        np.asarray(Wq, np.float32).T[:, perm].astype(_mld.bfloat16))
    WkvT = np.ascontiguousarray(np.concatenate(
        [np.asarray(Wk, np.float32).T, np.asarray(Wv, np.float32).T],
        axis=1).astype(_mld.bfloat16))
    WoT = np.ascontiguousarray(
        np.asarray(Wo, np.float32).T[perm, :].astype(_mld.bfloat16))
    cosT = np.asarray(cos, np.float32).T        # (64, T)
    sinT = np.asarray(sin, np.float32).T
    # 128-row rope tables: row r uses hd-dim r%64; sin rows sign-folded
    # (-sin for (r%64)<32) so rope is x*cos2 + swap32(x)*sin2 on 128 rows.
    sgn = np.where(np.arange(HD) < HD // 2, -1.0, 1.0).astype(np.float32)
    sgnc = sgn  # same sign-fold along the free axis for (t, d) layout
    cos2 = np.ascontiguousarray(np.tile(cosT, (2, 1)))              # (128, T)
    sin2 = np.ascontiguousarray(np.tile(sinT * sgn[:, None], (2, 1)))
    ones = np.ones((1, HD), np.float32)
    maps = []
    for c in range(8):
        b, j = c // 4, c % 4
        qb = _qblocks(j)
        cols = np.concatenate([np.arange(P * g, P * g + P) for g in qb])
        mask = np.empty((NB, P, P), _mld.bfloat16)
        ki = np.arange(P)[:, None]
        qi = np.arange(P)[None, :]
        for kb in range(NB):
            qg = qb[kb // 4]
            mask[kb] = np.where(P * kb + ki <= P * qg + qi, 0.0, 1.0)
        negi_np = (np.eye(P, dtype=np.float32) *
                   np.float32(-2.0 ** 30)).astype(_mld.bfloat16)
        maps.append({
            "xtq": np.ascontiguousarray(
                xT[b][:, cols].astype(_mld.bfloat16)),
            "xtv": np.ascontiguousarray(
                xT[b][:, 512 * j:512 * j + 512].astype(_mld.bfloat16)),
            "wqt": WqT,
            "wkvt": WkvT,
            "wot": WoT,
            "costq": np.ascontiguousarray(cos2[:, cols]),
            "sintq": np.ascontiguousarray(sin2[:, cols]),
            "costv8": np.ascontiguousarray(
                np.tile(cosT.T[512 * j:512 * j + 512, :], (1, KV))),
            "sintv8": np.ascontiguousarray(
                np.tile(sinT.T[512 * j:512 * j + 512, :] * sgnc[None, :],
                        (1, KV))),
            "ident": np.eye(P, dtype=np.float32),
            "masku": mask,
            "negi": negi_np,
            "onesr": ones,
        })
    return maps


def kernel(x, cos, sin, Wq, Wk, Wv, Wo):
    nc = _get_nc()
    maps = _in_maps(x, cos, sin, Wq, Wk, Wv, Wo)
    res = run_bass_kernel_spmd(nc, maps, list(range(8)))
    out = np.empty((B, T, D), np.float32)
    for c in range(8):
        b, j = c // 4, c % 4
        yc = res.results[c]["y"]
        for s, qg in enumerate(_qblocks(j)):
            out[b, P * qg:P * qg + P, :] = yc[P * s:P * s + P, :]
    return out


# revision 59
# speedup vs baseline: 1154.9972x; 1.0051x over previous
"""Bass/Trainium2 SPMD kernel for GQA causal attention with RoPE.

Sharding (8 cores): core c = 4*b + j (b = batch, j = 0..3 shard in batch).
  - Q / attention / o_proj: token-sharded; core j owns q-token 128-blocks
    {j, 7-j, j+8, 15-j} (balanced causal work, uniform SPMD program with
    fixed per-slot key extents [512, 1024, 1536, 2048] and data-driven masks).
  - K and V: token-sharded (core j computes tokens [512j, 512j+512), all
    dims; K roped in (t, d) layout). Split AllGathers (K first, then V, both
    bf16) assemble full K and V; K is PE-transposed on chip to (d, t) tiles.
  - Attention in s^T = (kpos, q) layout: scores = k^T.T @ q^T, exp on ACT
    (scale folds 1/sqrt(hd)), AV with ones-augmented V gives softmax sums,
    division applied after AV (commutes with the linear AV/o_proj steps).

v2 structure (cost-model driven):
  - bf16 q/k/v/p operand path: every attention matmul runs 1 cycle/row at
    any free size (fp32r pays 4x below 256), and the K/V AllGather traffic
    halves.
  - exp batching: scores for 2 kb per PSUM group tile ([128,1024] = 2 banks
    for kb 0-7; bank-packed [128,512] tiles for kb 8-15 exploiting the
    PSUM zero-region overwrite semantics) -> 7 exps/head instead of 16.
  - o_proj runs interleaved with attention: after each quad of otr blocks
    completes, its 64 matmuls + DVE adds into bf16 ysb partials are spread
    across the following heads' PE stream (PSUM slots shared with oaug/pb
    via a common pool tag).
  - vaug assembled by 4D-AP DMAs straight from the gathered V (no ACT/DVE
    copies); division output written directly into otr SBUF tiles (no DRAM
    round-trip through obuf).
  - DMA issue split: SP carries the load streams, GPSIMD/SWDGE carries the
    collective-adjacent stores + unpack (25ns issue vs 565ns, and no HWDGE
    contention).
"""
import numpy as np

import concourse.bass as bass
import concourse.tile as tile
from concourse import bacc, mybir
from concourse.bass_utils import run_bass_kernel_spmd

B, T, D = 2, 2048, 2048
H, KV, HD = 32, 8, 64
P = 128
NB = T // P          # 16 token blocks of 128
OWN = 4 * P          # 512 owned q tokens per core
f32 = mybir.dt.float32
f32r = mybir.dt.float32r
bf16 = mybir.dt.bfloat16
Exp = mybir.ActivationFunctionType.Exp
fp8 = mybir.dt.float8e4
DR = mybir.MatmulPerfMode.DoubleRow

AG_K = P * T           # 262144 fp8 elems of folded kT shard
VSH = 4 * KV * (HD + 1)  # 2080 v-shard columns (vaug layout, ones included)
AG_V = P * VSH         # 266240 elems of v shard
VROW = NB * (HD + 1)   # vaug length per kv head in (kb, kh) indexing

# attention group plan: (kbs, kind); kind 2 = [P,1024] two-bank tile with
# one kb per bank, kind 1 = [P,512] single bank packed with 2 or 4 kbs
GROUPS = [((0, 1), 2), ((2, 3), 2), ((4, 5), 2), ((6, 7), 2),
          ((8, 9), 1), ((10, 11), 1), ((12, 13, 14, 15), 1)]


def _qblocks(j):
    return [j, 7 - j, j + 8, 15 - j]


def _build(repeat=1, collective=True):
    nc = bacc.Bacc("TRN2", target_bir_lowering=False, debug=False, num_devices=8)

    xtq = nc.dram_tensor("xtq", [D, OWN], bf16, kind="ExternalInput").ap()
    xtv = nc.dram_tensor("xtv", [D, 512], bf16, kind="ExternalInput").ap()
    wqt = nc.dram_tensor("wqt", [D, H * HD], bf16, kind="ExternalInput").ap()
    wkvt = nc.dram_tensor("wkvt", [D, 2 * KV * HD], bf16,
                          kind="ExternalInput").ap()
    costv8 = nc.dram_tensor("costv8", [512, 512], f32, kind="ExternalInput").ap()
    sintv8 = nc.dram_tensor("sintv8", [512, 512], f32, kind="ExternalInput").ap()
    ident = nc.dram_tensor("ident", [P, P], f32, kind="ExternalInput").ap()
    wot = nc.dram_tensor("wot", [H * HD, D], bf16, kind="ExternalInput").ap()
    costq = nc.dram_tensor("costq", [P, OWN], f32, kind="ExternalInput").ap()
    sintq = nc.dram_tensor("sintq", [P, OWN], f32, kind="ExternalInput").ap()
    masku = nc.dram_tensor("masku", [NB, P, P], bf16, kind="ExternalInput").ap()
    negi = nc.dram_tensor("negi", [P, P], bf16, kind="ExternalInput").ap()
    onesr = nc.dram_tensor("onesr", [1, HD], f32, kind="ExternalInput").ap()
    y = nc.dram_tensor("y", [OWN, D], f32, kind="ExternalOutput").ap()

    def rope_full(dst, src, cosr, sinr, tmp_pool):
        """dst = bf16(src*cos + swap32(src)*sin_signed) over 128 rows.

        cosr rows r = cos[r%64]; sinr rows are sign-folded (-sin for
        (r%64)<32, +sin otherwise). The 32-row half-swaps run on the scalar
        engine (idle during projections); the wide elementwise ops run on
        DVE at full lane utilization."""
        xr = tmp_pool.tile([P, 512], f32, tag="xrot", bufs=4, name="xr")
        for po in (0, 64):
            nc.scalar.copy(xr[po:po + 32, :], src[po + 32:po + 64, :])
            nc.scalar.copy(xr[po + 32:po + 64, :], src[po:po + 32, :])
        u = tmp_pool.tile([P, 512], f32, tag="ropeu", bufs=4, name="u")
        v = tmp_pool.tile([P, 512], f32, tag="ropev", bufs=4, name="v")
        nc.vector.tensor_mul(u[:], src[:], cosr)
        nc.vector.tensor_mul(v[:], xr[:], sinr)
        nc.vector.tensor_add(dst[:], u[:], v[:])

    with tile.TileContext(nc) as tc:
        _dpool_cm = tc.tile_pool(name="dram", bufs=1, space="DRAM")
        dpool = _dpool_cm.__enter__()
        _pers_cm = tc.tile_pool(name="pers", bufs=1)
        pers = _pers_cm.__enter__()

        agink = dpool.tile([AG_K], bf16, tag="agink", name="agink")
        aginv = dpool.tile([AG_V], bf16, tag="aginv", name="aginv")
        agoutk = dpool.tile([4, AG_K], bf16, tag="agoutk", name="agoutk")
        agoutv = dpool.tile([4, AG_V], bf16, tag="agoutv", name="agoutv")

        # persistent across stages
        cosq_t = pers.tile([P, OWN], f32, tag="cosq_t", name="cosq_t")
        sinq_t = pers.tile([P, OWN], f32, tag="sinq_t", name="sinq_t")
        ones_t = pers.tile([1, HD], f32r, tag="ones_t", name="ones_t")
        mask_t = pers.tile([P, NB * P], bf16, tag="mask_t", name="mask_t")
        negi_t = pers.tile([P, P], bf16, tag="negi_t", name="negi_t")
        qtr = [pers.tile([P, OWN], bf16, tag=f"qtr{i}", name=f"qtr{i}")
               for i in range(16)]
        # ktr_all[p, db*2048 + t] = k^T[128*db + p, t]
        ktr_all = pers.tile([P, 4 * T], bf16, tag="ktr_all", name="ktr_all")
        vaug = pers.tile([P, KV * VROW], bf16, tag="vaug", name="vaug")

        def emit_body():
         # ================= stages A-D: projections + AllGather ============
         with tc.tile_pool(name="consA", bufs=1) as cA, \
             tc.tile_pool(name="wk", bufs=1) as wkp, \
             tc.tile_pool(name="wst", bufs=4) as wst, \
             tc.tile_pool(name="xs", bufs=4) as xsp, \
             tc.tile_pool(name="tmp", bufs=4) as tmpp, \
             tc.tile_pool(name="cpy", bufs=3) as cpyp, \
             tc.tile_pool(name="pproj", bufs=1, space="PSUM") as pproj:

            # ---- stages A+B: token-sharded K and V projections ----
            costd = cA.tile([P, 4 * 512], f32, tag="costd", name="costd")
            sintd = cA.tile([P, 4 * 512], f32, tag="sintd", name="sintd")
            idtA = cA.tile([P, P], bf16, tag="idtA", name="idtA")
            idf = cA.tile([P, P], f32, tag="idf", name="idf")
            krT_f = cA.tile([P, 4 * 512], bf16, tag="krT_f", name="krT_f")

            psk = [pproj.tile([P, 512], f32, tag=f"k{i}", bufs=1,
                              name=f"psk{i}") for i in range(4)]
            psv = [pproj.tile([P, 512], f32, tag=f"v{i}", bufs=1,
                              name=f"psv{i}") for i in range(4)]
            xtq_sb = [wkp.tile([P, OWN], bf16, tag=f"xq{cb}", name=f"xq{cb}")
                      for cb in range(16)]
            wqf = [wkp.tile([P, H * HD], bf16, tag=f"wqf{cb}",
                            name=f"wqf{cb}") for cb in range(16)]
            for cb in range(16):
                wkv_t = wst.tile([P, 2 * KV * HD], bf16, tag="wkv",
                                 name="wkv")
                nc.sync.dma_start(wkv_t[:], wkvt[P * cb:P * cb + P, :])
                wk_t = wkv_t[:, 0:KV * HD]
                wv_t = wkv_t[:, KV * HD:2 * KV * HD]
                xv_t = xsp.tile([P, 512], bf16, tag="xtv", name="xtv")
                nc.sync.dma_start(
                    xv_t[:], xtv[P * cb:P * cb + P, :])
                # prefetch streams ride SWDGE (Pool is idle here) so the
                # hot KV loads keep HWDGE to themselves
                nc.gpsimd.dma_start(xtq_sb[cb][:], xtq[P * cb:P * cb + P, :])
                nc.sync.dma_start(wqf[cb][:], wqt[P * cb:P * cb + P, :])
                if cb < 4:
                    nc.gpsimd.dma_start(costd[:, 512 * cb:512 * cb + 512],
                                        costv8[P * cb:P * cb + P, :])
                    nc.gpsimd.dma_start(sintd[:, 512 * cb:512 * cb + 512],
                                        sintv8[P * cb:P * cb + P, :])
                elif cb == 4:
                    nc.gpsimd.dma_start(idf[:], ident[:])
                    nc.scalar.copy(idtA[:], idf[:])
                    nc.gpsimd.dma_start(negi_t[:], negi[:])
                    nc.gpsimd.dma_start(ones_t[:], onesr[:].bitcast(f32r))
                elif cb == 5:
                    nc.gpsimd.dma_start(cosq_t[:], costq[:])
                    nc.gpsimd.dma_start(sinq_t[:], sintq[:])
                elif cb == 6:
                    nc.gpsimd.dma_start(
                        mask_t.rearrange("p (b c) -> p b c", b=NB),
                        masku.rearrange("b p c -> p b c"))
                for vb in range(4):
                    nc.tensor.matmul(psk[vb][:],
                                     lhsT=xv_t[:, P * vb:P * vb + P],
                                     rhs=wk_t,
                                     start=(cb == 0), stop=(cb == 15))
                    nc.tensor.matmul(psv[vb][:],
                                     lhsT=xv_t[:, P * vb:P * vb + P],
                                     rhs=wv_t,
                                     start=(cb == 0), stop=(cb == 15))
            # V shard assembled in vaug layout locally (strided ACT copies +
            # ones columns), then one DMA out
            vs_big = cA.tile([P, VSH], bf16, tag="vs_big", name="vs_big")
            vsv = vs_big.rearrange("p (b k c) -> p b k c", k=KV, c=HD + 1)
            nc.vector.memset(vsv[:, :, :, HD], 1.0)
            for vb in range(4):
                nc.scalar.copy(
                    vsv[:, vb, :, 0:HD],
                    psv[vb].rearrange("p (k c) -> p k c", k=KV))
            nc.gpsimd.dma_start(
                aginv.rearrange("(p c) -> p c", p=P), vs_big[:])
            # K rope in (t, d) layout; transpose to (d, t); write shard
            for vb in range(4):
                cs = costd[:, 512 * vb:512 * vb + 512]
                sn = sintd[:, 512 * vb:512 * vb + 512]
                u = tmpp.tile([P, 512], f32, tag="ropeu", bufs=4, name="u")
                nc.vector.tensor_mul(u[:], psk[vb][:], cs)
                vv = tmpp.tile([P, 512], f32, tag="ropev", bufs=4, name="vv")
                pr = psk[vb].rearrange("p (h s w) -> p h s w", s=2, w=32)
                vr = vv.rearrange("p (h s w) -> p h s w", s=2, w=32)
                sr = sn.rearrange("p (h s w) -> p h s w", s=2, w=32)
                nc.vector.tensor_mul(vr[:, :, 0, :], pr[:, :, 1, :],
                                     sr[:, :, 0, :])
                nc.vector.tensor_mul(vr[:, :, 1, :], pr[:, :, 0, :],
                                     sr[:, :, 1, :])
                kr = cpyp.tile([P, 512], bf16, tag="kr", name="kr")
                nc.vector.tensor_add(kr[:], u[:], vv[:])
                for db in range(4):
                    ptr = pproj.tile([P, P], bf16, tag=f"v{db}", bufs=1,
                                     name="ptrA")
                    nc.tensor.transpose(ptr[:], kr[:, P * db:P * db + P],
                                        idtA[:])
                    dst = krT_f[:, 512 * db + P * vb:512 * db + P * vb + P]
                    if db % 2 == 0:
                        nc.vector.tensor_copy(dst, ptr[:])
                    else:
                        nc.scalar.copy(dst, ptr[:])
            nc.gpsimd.dma_start(
                agink.rearrange("(p c) -> p c", p=P), krT_f[:])

            # ---- stage C: split AllGathers (V first — ready earlier) ----
            if collective:
                nc.gpsimd.collective_compute(
                    "AllGather",
                    mybir.AluOpType.bypass,
                    replica_groups=[[0, 1, 2, 3], [4, 5, 6, 7]],
                    ins=[aginv.opt()],
                    outs=[agoutv.opt()],
                )
                nc.gpsimd.collective_compute(
                    "AllGather",
                    mybir.AluOpType.bypass,
                    replica_groups=[[0, 1, 2, 3], [4, 5, 6, 7]],
                    ins=[agink.opt()],
                    outs=[agoutk.opt()],
                )
            else:
                for g in range(4):
                    nc.gpsimd.dma_start(
                        agoutv[g].rearrange("(a b) -> a b", b=8320),
                        aginv.rearrange("(a b) -> a b", b=8320))
                for g in range(4):
                    nc.gpsimd.dma_start(
                        agoutk[g].rearrange("(a b) -> a b", b=8192),
                        agink.rearrange("(a b) -> a b", b=8192))

            # ---- unpack: both in-gather layouts need 4 DMAs each ----
            for g in range(4):
                ksrc = agoutk[g, 0:AG_K].rearrange(
                    "(p d t) -> p d t", p=P, t=512)
                kdst = ktr_all.rearrange("p (d t) -> p d t", d=4)[
                    :, :, 512 * g:512 * g + 512]
                nc.gpsimd.dma_start(kdst, ksrc)
            for g in range(4):
                nc.gpsimd.dma_start(
                    vaug[:, VSH * g:VSH * g + VSH],
                    agoutv[g, 0:AG_V].rearrange("(p c) -> p c", p=P))

            # ---- stage D: Q projection (owned tokens) + rope ----
            for qg in range(4):
                tg = "k" if qg % 2 == 0 else "v"
                psq = [pproj.tile([P, 512], f32, tag=f"{tg}{i}", bufs=1,
                                  name=f"psq{i}") for i in range(4)]
                for cb in range(16):
                    wq_t = wqf[cb][:, 512 * qg:512 * qg + 512]
                    for qi in range(4):
                        nc.tensor.matmul(
                            psq[qi][:], lhsT=wq_t[:, P * qi:P * qi + P],
                            rhs=xtq_sb[cb], start=(cb == 0), stop=(cb == 15))
                for qi in range(4):
                    rope_full(qtr[4 * qg + qi], psq[qi], cosq_t[:],
                              sinq_t[:], tmpp)

         # ================= stages E-G: attention + fused o_proj ===========
         with tc.tile_pool(name="psS2", bufs=1, space="PSUM") as psS2, \
             tc.tile_pool(name="psS1", bufs=1, space="PSUM") as psS1, \
             tc.tile_pool(name="psO", bufs=1, space="PSUM") as psO, \
             tc.tile_pool(name="ptp", bufs=4) as ptp, \
             tc.tile_pool(name="nrm", bufs=3) as nrm, \
             tc.tile_pool(name="wos", bufs=1) as wos, \
             tc.tile_pool(name="otrg", bufs=1) as otrp, \
             tc.tile_pool(name="ysbp", bufs=1) as ysbp, \
             tc.tile_pool(name="yc", bufs=2) as ycp:

            otr = [otrp.tile([P, OWN], bf16, tag=f"otr{i % 8}",
                             name=f"otr{i}") for i in range(16)]
            ysb = {}
            for eb in range(4):
                for tb in range(4):
                    ysb[(eb, tb)] = ysbp.tile(
                        [P, 512], bf16, tag=f"ysb{eb}_{tb}",
                        name=f"ysb{eb}_{tb}")

            # wo chunk prefetch: o_proj contraction chunk ci covers
            # ab in CHUNKS[ci]; prefetched two chunks ahead.
            CHUNKS = [(0, 4), (4, 8), (8, 12), (12, 14), (14, 16)]
            wo_ch = {}

            def prefetch_chunk(ci):
                for ab in range(*CHUNKS[ci]):
                    for eb in range(4):
                        wt = wos.tile([P, 512], bf16, tag=f"wo{ab % 8}_{eb}",
                                      name=f"wo{ab}_{eb}", bufs=1)
                        nc.sync.dma_start(
                            wt[:], wot[P * ab:P * ab + P,
                                       512 * eb:512 * eb + 512])
                        wo_ch[(eb, ab)] = wt

            prefetch_chunk(0)
            prefetch_chunk(1)

            # o_proj work queue: (q, eb, tb) quanta emitted between attention
            # groups to fill the PE; the PSUM slot tag is picked at pop time
            # (the score tags join the rotation once attention has drained).
            oproj_work = []

            def enqueue_chunk(ci):
                for eb in range(4):
                    for tb in range(4):
                        oproj_work.append((ci, eb, tb))

            def emit_oproj(ci, eb, tb, tag):
                lo, hi = CHUNKS[ci]
                pool = psO if tag == "oaug" else (
                    psS1 if tag == "st1" else psS2)
                psg = pool.tile([P, 512], f32, tag=tag, bufs=2, name="psg")
                for ab in range(lo, hi):
                    nc.tensor.matmul(
                        psg[:], lhsT=otr[ab][:, P * tb:P * tb + P],
                        rhs=wo_ch[(eb, ab)],
                        start=(ab == lo), stop=(ab == hi - 1))
                t = ysb[(eb, tb)]
                if ci == 0:
                    nc.vector.tensor_copy(t[:], psg[:])
                elif ci < len(CHUNKS) - 1:
                    nc.vector.tensor_add(t[:], t[:], psg[:])
                else:
                    yt = ycp.tile([P, 512], f32, tag="yt", bufs=6,
                                  name="yt")
                    nc.vector.tensor_add(yt[:], t[:], psg[:])
                    eng = nc.sync if (eb + tb) % 2 == 0 else nc.gpsimd
                    eng.dma_start(
                        y[P * tb:P * tb + P, 512 * eb:512 * eb + 512], yt[:])

            def pop_oproj(k, tags=("oaug",)):
                for i in range(min(k, len(oproj_work))):
                    ci, eb, tb = oproj_work.pop(0)
                    emit_oproj(ci, eb, tb, tags[i % len(tags)])

            def emit_group(st, pt, kbs, kind, w, v4, m, side):
                qs = P * (kbs[0] // 4)
                n = OWN - qs
                kpo = HD * side
                kt = ktr_all[kpo:kpo + HD,
                             2048 * v4:2048 * v4 + 2048]
                rhs = qtr[4 * v4 + m][kpo:kpo + HD, qs:OWN]
                for i, kb in enumerate(kbs):
                    nc.tensor.matmul(
                        st[:, w * i:w * i + n],
                        lhsT=kt[:, P * kb:P * kb + P],
                        rhs=rhs,
                        start=(kind == 2 or i == 0), stop=False,
                        skip_group_check=True)
                if len(kbs) == 4:
                    nc.tensor.matmul(
                        st[:, 0:512], lhsT=negi_t[:],
                        rhs=mask_t[:, P * kbs[0]:P * kbs[0] + 512],
                        start=False, stop=True, skip_group_check=True)
                else:
                    for i, kb in enumerate(kbs):
                        nc.tensor.matmul(
                            st[:, w * i:w * i + P], lhsT=negi_t[:],
                            rhs=mask_t[:, P * kb:P * kb + P],
                            start=False, stop=(i == len(kbs) - 1),
                            skip_group_check=True)
                # one exp per group (strided across banks for kind 2)
                if kind == 2 and n < 512:
                    stv = st.rearrange("p (s c) -> p s c", s=2)
                    ptv = pt.rearrange("p (s c) -> p s c", s=2)
                    nc.scalar.activation(ptv[:, :, 0:n], stv[:, :, 0:n],
                                         Exp, scale=0.125)
                else:
                    m = w * (len(kbs) - 1) + n
                    nc.scalar.activation(pt[:, 0:m], st[:, 0:m], Exp,
                                         scale=0.125)

            for uu in range(16):
                v4 = uu // 4
                # head pair interleaved at group granularity: tile uu rows
                # 0:64 = head a (kv 2*v4), rows 64:128 = head a+4 (2*v4+1)
                m_u = uu % 4
                oaug = [psO.tile([P, OWN], f32, tag="oaug", bufs=2,
                                 name=f"oaug{s_}") for s_ in range(2)]
                pend = {0: [], 1: []}  # per side, AV lag 2
                for kbs, kind in GROUPS:
                    for side in range(2):
                        kh = 2 * v4 + side
                        if kind == 2:
                            st = psS2.tile([P, 1024], f32, tag="st2",
                                           bufs=2, name="st2")
                            w = 512
                        else:
                            st = psS1.tile([P, 512], f32, tag="st1",
                                           bufs=2, name="st1")
                            w = 512 // len(kbs)
                        pt = ptp.tile([P, 1024], bf16, tag="pt", bufs=10,
                                      name="pt")
                        emit_group(st, pt, kbs, kind, w, v4, m_u, side)
                        pend[side].append((pt, kbs, w))
                        if len(pend[side]) > 2:
                            _emit_avs(nc, pend[side].pop(0), vaug,
                                      oaug[side], kh)
                for side in range(2):
                    for pe_ in pend[side]:
                        _emit_avs(nc, pe_, vaug, oaug[side], 2 * v4 + side)
                for side in range(2):
                    # normalization: rec = 1/sums; Pool broadcasts to 64 rows
                    rec = nrm.tile([1, OWN], f32, tag="rec", name="rec")
                    nc.vector.reciprocal(rec[:], oaug[side][HD:HD + 1, :])
                    pbs = nrm.tile([HD, OWN], f32, tag="pbs", bufs=3,
                                   name="pbs")
                    nc.gpsimd.partition_broadcast(pbs[:], rec[:])
                    nc.vector.tensor_mul(
                        otr[uu][HD * side:HD * side + HD, :],
                        oaug[side][0:HD, :], pbs[:])
                pop_oproj(4, tags=("st1",))
                ci = {3: 0, 7: 1, 11: 2, 13: 3, 15: 4}.get(uu)
                if ci is not None:
                    enqueue_chunk(ci)
                    if ci + 2 < len(CHUNKS):
                        prefetch_chunk(ci + 2)
            pop_oproj(len(oproj_work), tags=("oaug", "st1", "st2"))

        for _rep in range(repeat):
            emit_body()

        _pers_cm.__exit__(None, None, None)
        _dpool_cm.__exit__(None, None, None)

    nc.compile()
    return nc


def _emit_avs(nc, pend, vaug, oaug, kh):
    pt, kbs, w = pend
    qs = P * (kbs[0] // 4)
    n = OWN - qs
    for i, kb in enumerate(kbs):
        base = (HD + 1) * (KV * kb + kh)
        nc.tensor.matmul(
            oaug[0:HD + 1, qs:OWN],
            lhsT=vaug[:, base:base + HD + 1],
            rhs=pt[:, w * i:w * i + n],
            start=(kb == 0), stop=(kb == NB - 1))


_NC = None


def _get_nc():
    global _NC
    if _NC is None:
        _NC = _build()
    return _NC


def _head_perm():
    """Pair each even-kv head with its odd-kv partner (+4) in one 128-dim
    block, so q partition parity matches the kv head parity in kT tiles."""
    order = []
    for u in range(16):
        a = 8 * (u // 4) + u % 4
        for h in (a, a + 4):
            order.extend(range(HD * h, HD * h + HD))
    return np.asarray(order)


def _in_maps(x, cos, sin, Wq, Wk, Wv, Wo):
    xT = np.ascontiguousarray(np.transpose(np.asarray(x, np.float32), (0, 2, 1)))
    perm = _head_perm()
    import ml_dtypes as _mld
    WqT = np.ascontiguousarray(
        np.asarray(Wq, np.float32).T[:, perm].astype(_mld.bfloat16))
    WkvT = np.ascontiguousarray(np.concatenate(
        [np.asarray(Wk, np.float32).T, np.asarray(Wv, np.float32).T],
        axis=1).astype(_mld.bfloat16))
    WoT = np.ascontiguousarray(
        np.asarray(Wo, np.float32).T[perm, :].astype(_mld.bfloat16))
    cosT = np.asarray(cos, np.float32).T        # (64, T)
    sinT = np.asarray(sin, np.float32).T
    # 128-row rope tables: row r uses hd-dim r%64; sin rows sign-folded
    # (-sin for (r%64)<32) so rope is x*cos2 + swap32(x)*sin2 on 128 rows.
    sgn = np.where(np.arange(HD) < HD // 2, -1.0, 1.0).astype(np.float32)
    sgnc = sgn  # same sign-fold along the free axis for (t, d) layout
    cos2 = np.ascontiguousarray(np.tile(cosT, (2, 1)))              # (128, T)
    sin2 = np.ascontiguousarray(np.tile(sinT * sgn[:, None], (2, 1)))
    ones = np.ones((1, HD), np.float32)
    maps = []
    for c in range(8):
        b, j = c // 4, c % 4
        qb = _qblocks(j)
        cols = np.concatenate([np.arange(P * g, P * g + P) for g in qb])
        mask = np.empty((NB, P, P), _mld.bfloat16)
        ki = np.arange(P)[:, None]
        qi = np.arange(P)[None, :]
        for kb in range(NB):
            qg = qb[kb // 4]
            mask[kb] = np.where(P * kb + ki <= P * qg + qi, 0.0, 1.0)
        negi_np = (np.eye(P, dtype=np.float32) *
                   np.float32(-2.0 ** 30)).astype(_mld.bfloat16)
        maps.append({
            "xtq": np.ascontiguousarray(
                xT[b][:, cols].astype(_mld.bfloat16)),
            "xtv": np.ascontiguousarray(
                xT[b][:, 512 * j:512 * j + 512].astype(_mld.bfloat16)),
            "wqt": WqT,
            "wkvt": WkvT,
            "wot": WoT,
            "costq": np.ascontiguousarray(cos2[:, cols]),
            "sintq": np.ascontiguousarray(sin2[:, cols]),
            "costv8": np.ascontiguousarray(
                np.tile(cosT.T[512 * j:512 * j + 512, :], (1, KV))),
            "sintv8": np.ascontiguousarray(
                np.tile(sinT.T[512 * j:512 * j + 512, :] * sgnc[None, :],
                        (1, KV))),
            "ident": np.eye(P, dtype=np.float32),
            "masku": mask,
            "negi": negi_np,
            "onesr": ones,
        })
    return maps


def kernel(x, cos, sin, Wq, Wk, Wv, Wo):
    nc = _get_nc()
    maps = _in_maps(x, cos, sin, Wq, Wk, Wv, Wo)
    res = run_bass_kernel_spmd(nc, maps, list(range(8)))
    out = np.empty((B, T, D), np.float32)
    for c in range(8):
        b, j = c // 4, c % 4
        yc = res.results[c]["y"]
        for s, qg in enumerate(_qblocks(j)):
            out[b, P * qg:P * qg + P, :] = yc[P * s:P * s + P, :]
    return out


# revision 62
# speedup vs baseline: 1189.1894x; 1.0296x over previous
"""Bass/Trainium2 SPMD kernel for GQA causal attention with RoPE.

Sharding (8 cores): core c = 4*b + j (b = batch, j = 0..3 shard in batch).
  - Q / attention / o_proj: token-sharded; core j owns q-token 128-blocks
    {j, 7-j, j+8, 15-j} (balanced causal work, uniform SPMD program with
    fixed per-slot key extents [512, 1024, 1536, 2048] and data-driven masks).
  - K and V: token-sharded (core j computes tokens [512j, 512j+512), all
    dims; K roped in (t, d) layout). Split AllGathers (K first, then V, both
    bf16) assemble full K and V; K is PE-transposed on chip to (d, t) tiles.
  - Attention in s^T = (kpos, q) layout: scores = k^T.T @ q^T, exp on ACT
    (scale folds 1/sqrt(hd)), AV with ones-augmented V gives softmax sums,
    division applied after AV (commutes with the linear AV/o_proj steps).

v2 structure (cost-model driven):
  - bf16 q/k/v/p operand path: every attention matmul runs 1 cycle/row at
    any free size (fp32r pays 4x below 256), and the K/V AllGather traffic
    halves.
  - exp batching: scores for 2 kb per PSUM group tile ([128,1024] = 2 banks
    for kb 0-7; bank-packed [128,512] tiles for kb 8-15 exploiting the
    PSUM zero-region overwrite semantics) -> 7 exps/head instead of 16.
  - o_proj runs interleaved with attention: after each quad of otr blocks
    completes, its 64 matmuls + DVE adds into bf16 ysb partials are spread
    across the following heads' PE stream (PSUM slots shared with oaug/pb
    via a common pool tag).
  - vaug assembled by 4D-AP DMAs straight from the gathered V (no ACT/DVE
    copies); division output written directly into otr SBUF tiles (no DRAM
    round-trip through obuf).
  - DMA issue split: SP carries the load streams, GPSIMD/SWDGE carries the
    collective-adjacent stores + unpack (25ns issue vs 565ns, and no HWDGE
    contention).
"""
import numpy as np

import concourse.bass as bass
import concourse.tile as tile
from concourse import bacc, mybir
from concourse.bass_utils import run_bass_kernel_spmd

B, T, D = 2, 2048, 2048
H, KV, HD = 32, 8, 64
P = 128
NB = T // P          # 16 token blocks of 128
OWN = 4 * P          # 512 owned q tokens per core
f32 = mybir.dt.float32
f32r = mybir.dt.float32r
bf16 = mybir.dt.bfloat16
Exp = mybir.ActivationFunctionType.Exp
fp8 = mybir.dt.float8e4
DR = mybir.MatmulPerfMode.DoubleRow

AG_K = P * T           # 262144 fp8 elems of folded kT shard
VSH = 4 * KV * (HD + 1)  # 2080 v-shard columns (vaug layout, ones included)
AG_V = P * VSH         # 266240 elems of v shard
VROW = NB * (HD + 1)   # vaug length per kv head in (kb, kh) indexing

# attention group plan: (kbs, kind); kind 2 = [P,1024] two-bank tile with
# one kb per bank, kind 1 = [P,512] single bank packed with 2 or 4 kbs
GROUPS = [((0, 1), 2), ((2, 3), 2), ((4, 5), 2), ((6, 7), 2),
          ((8, 9), 1), ((10, 11), 1), ((12, 13, 14, 15), 1)]


def _qblocks(j):
    return [j, 7 - j, j + 8, 15 - j]


def _build(repeat=1, collective=True):
    nc = bacc.Bacc("TRN2", target_bir_lowering=False, debug=False, num_devices=8)

    xtq = nc.dram_tensor("xtq", [D, OWN], bf16, kind="ExternalInput").ap()
    xtv = nc.dram_tensor("xtv", [D, 512], bf16, kind="ExternalInput").ap()
    wqt = nc.dram_tensor("wqt", [D, H * HD], bf16, kind="ExternalInput").ap()
    wkvt = nc.dram_tensor("wkvt", [D, 2 * KV * HD], bf16,
                          kind="ExternalInput").ap()
    costv8 = nc.dram_tensor("costv8", [512, 512], f32, kind="ExternalInput").ap()
    sintv8 = nc.dram_tensor("sintv8", [512, 512], f32, kind="ExternalInput").ap()
    ident = nc.dram_tensor("ident", [P, P], f32, kind="ExternalInput").ap()
    wot = nc.dram_tensor("wot", [H * HD, D], bf16, kind="ExternalInput").ap()
    costq = nc.dram_tensor("costq", [P, OWN], f32, kind="ExternalInput").ap()
    sintq = nc.dram_tensor("sintq", [P, OWN], f32, kind="ExternalInput").ap()
    masku = nc.dram_tensor("masku", [NB, P, P], bf16, kind="ExternalInput").ap()
    negi = nc.dram_tensor("negi", [P, P], bf16, kind="ExternalInput").ap()
    onesr = nc.dram_tensor("onesr", [1, HD], f32, kind="ExternalInput").ap()
    y = nc.dram_tensor("y", [OWN, D], f32, kind="ExternalOutput").ap()

    def rope_full(dst, src, cosr, sinr, tmp_pool):
        """dst = bf16(src*cos + swap32(src)*sin_signed) over 128 rows.

        cosr rows r = cos[r%64]; sinr rows are sign-folded (-sin for
        (r%64)<32, +sin otherwise). The 32-row half-swaps run on the scalar
        engine (idle during projections); the wide elementwise ops run on
        DVE at full lane utilization."""
        xr = tmp_pool.tile([P, 512], f32, tag="xrot", bufs=4, name="xr")
        for po in (0, 64):
            nc.scalar.copy(xr[po:po + 32, :], src[po + 32:po + 64, :])
            nc.scalar.copy(xr[po + 32:po + 64, :], src[po:po + 32, :])
        u = tmp_pool.tile([P, 512], f32, tag="ropeu", bufs=4, name="u")
        v = tmp_pool.tile([P, 512], f32, tag="ropev", bufs=4, name="v")
        nc.vector.tensor_mul(u[:], src[:], cosr)
        nc.vector.tensor_mul(v[:], xr[:], sinr)
        nc.vector.tensor_add(dst[:], u[:], v[:])

    with tile.TileContext(nc) as tc:
        _dpool_cm = tc.tile_pool(name="dram", bufs=1, space="DRAM")
        dpool = _dpool_cm.__enter__()
        _pers_cm = tc.tile_pool(name="pers", bufs=1)
        pers = _pers_cm.__enter__()

        agink = dpool.tile([AG_K], bf16, tag="agink", name="agink")
        aginv = dpool.tile([AG_V], bf16, tag="aginv", name="aginv")
        agoutk = dpool.tile([4, AG_K], bf16, tag="agoutk", name="agoutk")
        agoutv = dpool.tile([4, AG_V], bf16, tag="agoutv", name="agoutv")

        # persistent across stages
        cosq_t = pers.tile([P, OWN], f32, tag="cosq_t", name="cosq_t")
        sinq_t = pers.tile([P, OWN], f32, tag="sinq_t", name="sinq_t")
        ones_t = pers.tile([1, HD], f32r, tag="ones_t", name="ones_t")
        mask_t = pers.tile([P, NB * P], bf16, tag="mask_t", name="mask_t")
        negi_t = pers.tile([P, P], bf16, tag="negi_t", name="negi_t")
        qtr = [pers.tile([P, OWN], bf16, tag=f"qtr{i}", name=f"qtr{i}")
               for i in range(16)]
        # ktr_all[p, db*2048 + t] = k^T[128*db + p, t]
        ktr_all = pers.tile([P, 4 * T], bf16, tag="ktr_all", name="ktr_all")
        vaug = pers.tile([P, KV * VROW], bf16, tag="vaug", name="vaug")

        def emit_body():
         # ================= stages A-D: projections + AllGather ============
         with tc.tile_pool(name="consA", bufs=1) as cA, \
             tc.tile_pool(name="wk", bufs=1) as wkp, \
             tc.tile_pool(name="wst", bufs=4) as wst, \
             tc.tile_pool(name="xs", bufs=4) as xsp, \
             tc.tile_pool(name="tmp", bufs=4) as tmpp, \
             tc.tile_pool(name="cpy", bufs=3) as cpyp, \
             tc.tile_pool(name="pproj", bufs=1, space="PSUM") as pproj:

            # ---- stages A+B: token-sharded K and V projections ----
            costd = cA.tile([P, 4 * 512], f32, tag="costd", name="costd")
            sintd = cA.tile([P, 4 * 512], f32, tag="sintd", name="sintd")
            idtA = cA.tile([P, P], bf16, tag="idtA", name="idtA")
            idf = cA.tile([P, P], f32, tag="idf", name="idf")
            krT_f = cA.tile([P, 4 * 512], bf16, tag="krT_f", name="krT_f")

            psk = [pproj.tile([P, 512], f32, tag=f"k{i}", bufs=1,
                              name=f"psk{i}") for i in range(4)]
            psv = [pproj.tile([P, 512], f32, tag=f"v{i}", bufs=1,
                              name=f"psv{i}") for i in range(4)]
            xtq_sb = [wkp.tile([P, OWN], bf16, tag=f"xq{cb}", name=f"xq{cb}")
                      for cb in range(16)]
            wqf = [wkp.tile([P, H * HD], bf16, tag=f"wqf{cb}",
                            name=f"wqf{cb}") for cb in range(16)]
            for cb in range(16):
                wkv_t = wst.tile([P, 2 * KV * HD], bf16, tag="wkv",
                                 name="wkv")
                nc.sync.dma_start(wkv_t[:], wkvt[P * cb:P * cb + P, :])
                wk_t = wkv_t[:, 0:KV * HD]
                wv_t = wkv_t[:, KV * HD:2 * KV * HD]
                xv_t = xsp.tile([P, 512], bf16, tag="xtv", name="xtv")
                nc.sync.dma_start(
                    xv_t[:], xtv[P * cb:P * cb + P, :])
                # prefetch streams ride SWDGE (Pool is idle here) so the
                # hot KV loads keep HWDGE to themselves
                nc.gpsimd.dma_start(xtq_sb[cb][:], xtq[P * cb:P * cb + P, :])
                nc.sync.dma_start(wqf[cb][:], wqt[P * cb:P * cb + P, :])
                if cb < 4:
                    nc.gpsimd.dma_start(costd[:, 512 * cb:512 * cb + 512],
                                        costv8[P * cb:P * cb + P, :])
                    nc.gpsimd.dma_start(sintd[:, 512 * cb:512 * cb + 512],
                                        sintv8[P * cb:P * cb + P, :])
                elif cb == 4:
                    nc.gpsimd.dma_start(idf[:], ident[:])
                    nc.scalar.copy(idtA[:], idf[:])
                    nc.gpsimd.dma_start(negi_t[:], negi[:])
                    nc.gpsimd.dma_start(ones_t[:], onesr[:].bitcast(f32r))
                elif cb == 5:
                    nc.gpsimd.dma_start(cosq_t[:], costq[:])
                    nc.gpsimd.dma_start(sinq_t[:], sintq[:])
                elif cb == 6:
                    nc.gpsimd.dma_start(
                        mask_t.rearrange("p (b c) -> p b c", b=NB),
                        masku.rearrange("b p c -> p b c"))
                for vb in range(4):
                    nc.tensor.matmul(psk[vb][:],
                                     lhsT=xv_t[:, P * vb:P * vb + P],
                                     rhs=wk_t,
                                     start=(cb == 0), stop=(cb == 15))
                    nc.tensor.matmul(psv[vb][:],
                                     lhsT=xv_t[:, P * vb:P * vb + P],
                                     rhs=wv_t,
                                     start=(cb == 0), stop=(cb == 15))
            # V shard assembled in vaug layout locally (strided ACT copies +
            # ones columns), then one DMA out
            vs_big = cA.tile([P, VSH], bf16, tag="vs_big", name="vs_big")
            vsv = vs_big.rearrange("p (b k c) -> p b k c", k=KV, c=HD + 1)
            nc.vector.memset(vsv[:, :, :, HD], 1.0)
            for vb in range(4):
                nc.scalar.copy(
                    vsv[:, vb, :, 0:HD],
                    psv[vb].rearrange("p (k c) -> p k c", k=KV))
            nc.gpsimd.dma_start(
                aginv.rearrange("(p c) -> p c", p=P), vs_big[:])
            # K rope in (t, d) layout; transpose to (d, t); write shard
            for vb in range(4):
                cs = costd[:, 512 * vb:512 * vb + 512]
                sn = sintd[:, 512 * vb:512 * vb + 512]
                u = tmpp.tile([P, 512], f32, tag="ropeu", bufs=4, name="u")
                nc.vector.tensor_mul(u[:], psk[vb][:], cs)
                vv = tmpp.tile([P, 512], f32, tag="ropev", bufs=4, name="vv")
                pr = psk[vb].rearrange("p (h s w) -> p h s w", s=2, w=32)
                vr = vv.rearrange("p (h s w) -> p h s w", s=2, w=32)
                sr = sn.rearrange("p (h s w) -> p h s w", s=2, w=32)
                nc.vector.tensor_mul(vr[:, :, 0, :], pr[:, :, 1, :],
                                     sr[:, :, 0, :])
                nc.vector.tensor_mul(vr[:, :, 1, :], pr[:, :, 0, :],
                                     sr[:, :, 1, :])
                kr = cpyp.tile([P, 512], bf16, tag="kr", name="kr")
                nc.vector.tensor_add(kr[:], u[:], vv[:])
                for db in range(4):
                    ptr = pproj.tile([P, P], bf16, tag=f"v{db}", bufs=1,
                                     name="ptrA")
                    nc.tensor.transpose(ptr[:], kr[:, P * db:P * db + P],
                                        idtA[:])
                    dst = krT_f[:, 512 * db + P * vb:512 * db + P * vb + P]
                    if db % 2 == 0:
                        nc.vector.tensor_copy(dst, ptr[:])
                    else:
                        nc.scalar.copy(dst, ptr[:])
            nc.gpsimd.dma_start(
                agink.rearrange("(p c) -> p c", p=P), krT_f[:])

            # ---- stage C: split AllGathers (V first — ready earlier) ----
            if collective:
                nc.gpsimd.collective_compute(
                    "AllGather",
                    mybir.AluOpType.bypass,
                    replica_groups=[[0, 1, 2, 3], [4, 5, 6, 7]],
                    ins=[aginv.opt()],
                    outs=[agoutv.opt()],
                )
                nc.gpsimd.collective_compute(
                    "AllGather",
                    mybir.AluOpType.bypass,
                    replica_groups=[[0, 1, 2, 3], [4, 5, 6, 7]],
                    ins=[agink.opt()],
                    outs=[agoutk.opt()],
                )
            else:
                for g in range(4):
                    nc.gpsimd.dma_start(
                        agoutv[g].rearrange("(a b) -> a b", b=8320),
                        aginv.rearrange("(a b) -> a b", b=8320))
                for g in range(4):
                    nc.gpsimd.dma_start(
                        agoutk[g].rearrange("(a b) -> a b", b=8192),
                        agink.rearrange("(a b) -> a b", b=8192))

            # ---- unpack: both in-gather layouts need 4 DMAs each ----
            for g in range(4):
                ksrc = agoutk[g, 0:AG_K].rearrange(
                    "(p d t) -> p d t", p=P, t=512)
                kdst = ktr_all.rearrange("p (d t) -> p d t", d=4)[
                    :, :, 512 * g:512 * g + 512]
                nc.gpsimd.dma_start(kdst, ksrc)
            for g in range(4):
                nc.gpsimd.dma_start(
                    vaug[:, VSH * g:VSH * g + VSH],
                    agoutv[g, 0:AG_V].rearrange("(p c) -> p c", p=P))

            # ---- stage D: Q projection (owned tokens) + rope ----
            for qg in range(4):
                tg = "k" if qg % 2 == 0 else "v"
                psq = [pproj.tile([P, 512], f32, tag=f"{tg}{i}", bufs=1,
                                  name=f"psq{i}") for i in range(4)]
                for cb in range(16):
                    wq_t = wqf[cb][:, 512 * qg:512 * qg + 512]
                    for qi in range(4):
                        nc.tensor.matmul(
                            psq[qi][:], lhsT=wq_t[:, P * qi:P * qi + P],
                            rhs=xtq_sb[cb], start=(cb == 0), stop=(cb == 15))
                for qi in range(4):
                    rope_full(qtr[4 * qg + qi], psq[qi], cosq_t[:],
                              sinq_t[:], tmpp)

         # ================= stages E-G: attention + fused o_proj ===========
         with tc.tile_pool(name="psS2", bufs=1, space="PSUM") as psS2, \
             tc.tile_pool(name="psS1", bufs=1, space="PSUM") as psS1, \
             tc.tile_pool(name="psO", bufs=1, space="PSUM") as psO, \
             tc.tile_pool(name="ptp", bufs=4) as ptp, \
             tc.tile_pool(name="nrm", bufs=3) as nrm, \
             tc.tile_pool(name="wos", bufs=1) as wos, \
             tc.tile_pool(name="otrg", bufs=1) as otrp, \
             tc.tile_pool(name="ysbp", bufs=1) as ysbp, \
             tc.tile_pool(name="yc", bufs=2) as ycp:

            otr = [otrp.tile([P, OWN], bf16, tag=f"otr{i % 8}",
                             name=f"otr{i}") for i in range(16)]
            ysb = {}
            for eb in range(4):
                for tb in range(4):
                    ysb[(eb, tb)] = ysbp.tile(
                        [P, 512], bf16, tag=f"ysb{eb}_{tb}",
                        name=f"ysb{eb}_{tb}")

            # wo chunk prefetch: o_proj contraction chunk ci covers
            # ab in CHUNKS[ci]; prefetched two chunks ahead.
            CHUNKS = [(0, 4), (4, 8), (8, 12), (12, 14), (14, 16)]
            wo_ch = {}

            def prefetch_chunk(ci):
                for ab in range(*CHUNKS[ci]):
                    for eb in range(4):
                        wt = wos.tile([P, 512], bf16, tag=f"wo{ab % 8}_{eb}",
                                      name=f"wo{ab}_{eb}", bufs=1)
                        nc.sync.dma_start(
                            wt[:], wot[P * ab:P * ab + P,
                                       512 * eb:512 * eb + 512])
                        wo_ch[(eb, ab)] = wt

            prefetch_chunk(0)
            prefetch_chunk(1)

            # o_proj work queue: (q, eb, tb) quanta emitted between attention
            # groups to fill the PE; the PSUM slot tag is picked at pop time
            # (the score tags join the rotation once attention has drained).
            oproj_work = []

            def enqueue_chunk(ci):
                for eb in range(4):
                    for tb in range(4):
                        oproj_work.append((ci, eb, tb))

            def emit_oproj(ci, eb, tb, tag):
                lo, hi = CHUNKS[ci]
                pool = psO if tag == "oaug" else psS2
                psg = pool.tile([P, 512], f32, tag=tag,
                                bufs=(2 if tag == "oaug" else 3), name="psg")
                for ab in range(lo, hi):
                    nc.tensor.matmul(
                        psg[:], lhsT=otr[ab][:, P * tb:P * tb + P],
                        rhs=wo_ch[(eb, ab)],
                        start=(ab == lo), stop=(ab == hi - 1))
                t = ysb[(eb, tb)]
                if ci == 0:
                    nc.vector.tensor_copy(t[:], psg[:])
                elif ci < len(CHUNKS) - 1:
                    nc.vector.tensor_add(t[:], t[:], psg[:])
                else:
                    yt = ycp.tile([P, 512], f32, tag="yt", bufs=6,
                                  name="yt")
                    nc.vector.tensor_add(yt[:], t[:], psg[:])
                    eng = nc.sync if (eb + tb) % 2 == 0 else nc.gpsimd
                    eng.dma_start(
                        y[P * tb:P * tb + P, 512 * eb:512 * eb + 512], yt[:])

            def pop_oproj(k, tags=("oaug",)):
                for i in range(min(k, len(oproj_work))):
                    ci, eb, tb = oproj_work.pop(0)
                    emit_oproj(ci, eb, tb, tags[i % len(tags)])

            def emit_group(st, pt, kbs, kind, w, v4, m, side):
                qs = P * (kbs[0] // 4)
                n = OWN - qs
                kpo = HD * side
                kt = ktr_all[kpo:kpo + HD,
                             2048 * v4:2048 * v4 + 2048]
                rhs = qtr[4 * v4 + m][kpo:kpo + HD, qs:OWN]
                for i, kb in enumerate(kbs):
                    nc.tensor.matmul(
                        st[:, w * i:w * i + n],
                        lhsT=kt[:, P * kb:P * kb + P],
                        rhs=rhs,
                        start=(kind == 2 or i == 0), stop=False,
                        skip_group_check=True)
                if len(kbs) == 4:
                    nc.tensor.matmul(
                        st[:, 0:512], lhsT=negi_t[:],
                        rhs=mask_t[:, P * kbs[0]:P * kbs[0] + 512],
                        start=False, stop=True, skip_group_check=True)
                else:
                    for i, kb in enumerate(kbs):
                        nc.tensor.matmul(
                            st[:, w * i:w * i + P], lhsT=negi_t[:],
                            rhs=mask_t[:, P * kb:P * kb + P],
                            start=False, stop=(i == len(kbs) - 1),
                            skip_group_check=True)
                # one exp per group (strided across banks for kind 2)
                if kind == 2 and n < 512:
                    stv = st.rearrange("p (s c) -> p s c", s=2)
                    ptv = pt.rearrange("p (s c) -> p s c", s=2)
                    nc.scalar.activation(ptv[:, :, 0:n], stv[:, :, 0:n],
                                         Exp, scale=0.125)
                else:
                    m = w * (len(kbs) - 1) + n
                    nc.scalar.activation(pt[:, 0:m], st[:, 0:m], Exp,
                                         scale=0.125)

            for uu in range(16):
                v4 = uu // 4
                # head pair interleaved at group granularity: tile uu rows
                # 0:64 = head a (kv 2*v4), rows 64:128 = head a+4 (2*v4+1)
                m_u = uu % 4
                oaug = [psO.tile([P, OWN], f32, tag="oaug", bufs=2,
                                 name=f"oaug{s_}") for s_ in range(2)]
                pend = {0: [], 1: []}  # per side, AV lag 2
                for kbs, kind in GROUPS:
                    for side in range(2):
                        kh = 2 * v4 + side
                        if kind == 2:
                            st = psS2.tile([P, 1024], f32, tag="st2",
                                           bufs=2, name="st2")
                            w = 512
                        else:
                            st = psS1.tile([P, 512], f32, tag="st1",
                                           bufs=2, name="st1")
                            w = 512 // len(kbs)
                        pt = ptp.tile([P, 1024], bf16, tag="pt", bufs=10,
                                      name="pt")
                        emit_group(st, pt, kbs, kind, w, v4, m_u, side)
                        pend[side].append((pt, kbs, w))
                        if len(pend[side]) > 2:
                            _emit_avs(nc, pend[side].pop(0), vaug,
                                      oaug[side], kh)
                for side in range(2):
                    for pe_ in pend[side]:
                        _emit_avs(nc, pe_, vaug, oaug[side], 2 * v4 + side)
                for side in range(2):
                    # normalization: rec = 1/sums; Pool broadcasts to 64 rows
                    rec = nrm.tile([1, OWN], f32, tag="rec", name="rec")
                    nc.vector.reciprocal(rec[:], oaug[side][HD:HD + 1, :])
                    pbs = nrm.tile([HD, OWN], f32, tag="pbs", bufs=3,
                                   name="pbs")
                    nc.gpsimd.partition_broadcast(pbs[:], rec[:])
                    nc.vector.tensor_mul(
                        otr[uu][HD * side:HD * side + HD, :],
                        oaug[side][0:HD, :], pbs[:])
                pop_oproj(4, tags=("st1",))
                ci = {3: 0, 7: 1, 11: 2, 13: 3, 15: 4}.get(uu)
                if ci is not None:
                    enqueue_chunk(ci)
                    if ci + 2 < len(CHUNKS):
                        prefetch_chunk(ci + 2)
            pop_oproj(len(oproj_work), tags=("oaug", "st1", "st2"))

        for _rep in range(repeat):
            emit_body()

        _pers_cm.__exit__(None, None, None)
        _dpool_cm.__exit__(None, None, None)

    nc.compile()
    return nc


def _emit_avs(nc, pend, vaug, oaug, kh):
    pt, kbs, w = pend
    qs = P * (kbs[0] // 4)
    n = OWN - qs
    for i, kb in enumerate(kbs):
        base = (HD + 1) * (KV * kb + kh)
        nc.tensor.matmul(
            oaug[0:HD + 1, qs:OWN],
            lhsT=vaug[:, base:base + HD + 1],
            rhs=pt[:, w * i:w * i + n],
            start=(kb == 0), stop=(kb == NB - 1))


_NC = None


def _get_nc():
    global _NC
    if _NC is None:
        _NC = _build()
    return _NC


def _head_perm():
    """Pair each even-kv head with its odd-kv partner (+4) in one 128-dim
    block, so q partition parity matches the kv head parity in kT tiles."""
    order = []
    for u in range(16):
        a = 8 * (u // 4) + u % 4
        for h in (a, a + 4):
            order.extend(range(HD * h, HD * h + HD))
    return np.asarray(order)


def _in_maps(x, cos, sin, Wq, Wk, Wv, Wo):
    xT = np.ascontiguousarray(np.transpose(np.asarray(x, np.float32), (0, 2, 1)))
    perm = _head_perm()
    import ml_dtypes as _mld
    WqT = np.ascontiguousarray(
        np.asarray(Wq, np.float32).T[:, perm].astype(_mld.bfloat16))
    WkvT = np.ascontiguousarray(np.concatenate(
        [np.asarray(Wk, np.float32).T, np.asarray(Wv, np.float32).T],
        axis=1).astype(_mld.bfloat16))
    WoT = np.ascontiguousarray(
        np.asarray(Wo, np.float32).T[perm, :].astype(_mld.bfloat16))
    cosT = np.asarray(cos, np.float32).T        # (64, T)
    sinT = np.asarray(sin, np.float32).T
    # 128-row rope tables: row r uses hd-dim r%64; sin rows sign-folded
    # (-sin for (r%64)<32) so rope is x*cos2 + swap32(x)*sin2 on 128 rows.
    sgn = np.where(np.arange(HD) < HD // 2, -1.0, 1.0).astype(np.float32)
    sgnc = sgn  # same sign-fold along the free axis for (t, d) layout
    cos2 = np.ascontiguousarray(np.tile(cosT, (2, 1)))              # (128, T)
    sin2 = np.ascontiguousarray(np.tile(sinT * sgn[:, None], (2, 1)))
    ones = np.ones((1, HD), np.float32)
    maps = []
    for c in range(8):
        b, j = c // 4, c % 4
        qb = _qblocks(j)
        cols = np.concatenate([np.arange(P * g, P * g + P) for g in qb])
        mask = np.empty((NB, P, P), _mld.bfloat16)
        ki = np.arange(P)[:, None]
        qi = np.arange(P)[None, :]
        for kb in range(NB):
            qg = qb[kb // 4]
            mask[kb] = np.where(P * kb + ki <= P * qg + qi, 0.0, 1.0)
        negi_np = (np.eye(P, dtype=np.float32) *
                   np.float32(-2.0 ** 30)).astype(_mld.bfloat16)
        maps.append({
            "xtq": np.ascontiguousarray(
                xT[b][:, cols].astype(_mld.bfloat16)),
            "xtv": np.ascontiguousarray(
                xT[b][:, 512 * j:512 * j + 512].astype(_mld.bfloat16)),
            "wqt": WqT,
            "wkvt": WkvT,
            "wot": WoT,
            "costq": np.ascontiguousarray(cos2[:, cols]),
            "sintq": np.ascontiguousarray(sin2[:, cols]),
            "costv8": np.ascontiguousarray(
                np.tile(cosT.T[512 * j:512 * j + 512, :], (1, KV))),
            "sintv8": np.ascontiguousarray(
                np.tile(sinT.T[512 * j:512 * j + 512, :] * sgnc[None, :],
                        (1, KV))),
            "ident": np.eye(P, dtype=np.float32),
            "masku": mask,
            "negi": negi_np,
            "onesr": ones,
        })
    return maps


def kernel(x, cos, sin, Wq, Wk, Wv, Wo):
    nc = _get_nc()
    maps = _in_maps(x, cos, sin, Wq, Wk, Wv, Wo)
    res = run_bass_kernel_spmd(nc, maps, list(range(8)))
    out = np.empty((B, T, D), np.float32)
    for c in range(8):
        b, j = c // 4, c % 4
        yc = res.results[c]["y"]
        for s, qg in enumerate(_qblocks(j)):
            out[b, P * qg:P * qg + P, :] = yc[P * s:P * s + P, :]
    return out


# revision 66
# speedup vs baseline: 1193.2109x; 1.0034x over previous
"""Bass/Trainium2 SPMD kernel for GQA causal attention with RoPE.

Sharding (8 cores): core c = 4*b + j (b = batch, j = 0..3 shard in batch).
  - Q / attention / o_proj: token-sharded; core j owns q-token 128-blocks
    {j, 7-j, j+8, 15-j} (balanced causal work, uniform SPMD program with
    fixed per-slot key extents [512, 1024, 1536, 2048] and data-driven masks).
  - K and V: token-sharded (core j computes tokens [512j, 512j+512), all
    dims; K roped in (t, d) layout). Split AllGathers (K first, then V, both
    bf16) assemble full K and V; K is PE-transposed on chip to (d, t) tiles.
  - Attention in s^T = (kpos, q) layout: scores = k^T.T @ q^T, exp on ACT
    (scale folds 1/sqrt(hd)), AV with ones-augmented V gives softmax sums,
    division applied after AV (commutes with the linear AV/o_proj steps).

v2 structure (cost-model driven):
  - bf16 q/k/v/p operand path: every attention matmul runs 1 cycle/row at
    any free size (fp32r pays 4x below 256), and the K/V AllGather traffic
    halves.
  - exp batching: scores for 2 kb per PSUM group tile ([128,1024] = 2 banks
    for kb 0-7; bank-packed [128,512] tiles for kb 8-15 exploiting the
    PSUM zero-region overwrite semantics) -> 7 exps/head instead of 16.
  - o_proj runs interleaved with attention: after each quad of otr blocks
    completes, its 64 matmuls + DVE adds into bf16 ysb partials are spread
    across the following heads' PE stream (PSUM slots shared with oaug/pb
    via a common pool tag).
  - vaug assembled by 4D-AP DMAs straight from the gathered V (no ACT/DVE
    copies); division output written directly into otr SBUF tiles (no DRAM
    round-trip through obuf).
  - DMA issue split: SP carries the load streams, GPSIMD/SWDGE carries the
    collective-adjacent stores + unpack (25ns issue vs 565ns, and no HWDGE
    contention).
"""
import numpy as np

import concourse.bass as bass
import concourse.tile as tile
from concourse import bacc, mybir
from concourse.bass_utils import run_bass_kernel_spmd

B, T, D = 2, 2048, 2048
H, KV, HD = 32, 8, 64
P = 128
NB = T // P          # 16 token blocks of 128
OWN = 4 * P          # 512 owned q tokens per core
f32 = mybir.dt.float32
f32r = mybir.dt.float32r
bf16 = mybir.dt.bfloat16
Exp = mybir.ActivationFunctionType.Exp
fp8 = mybir.dt.float8e4
DR = mybir.MatmulPerfMode.DoubleRow

AG_K = P * T           # 262144 fp8 elems of folded kT shard
VSH = 4 * KV * (HD + 1)  # 2080 v-shard columns (vaug layout, ones included)
AG_V = P * VSH         # 266240 elems of v shard
VROW = NB * (HD + 1)   # vaug length per kv head in (kb, kh) indexing

# attention group plan: (kbs, kind); kind 2 = [P,1024] two-bank tile with
# one kb per bank, kind 1 = [P,512] single bank packed with 2 or 4 kbs
GROUPS = [((0, 1), 2), ((2, 3), 2), ((4, 5), 2), ((6, 7), 2),
          ((8, 9), 1), ((10, 11), 1), ((12, 13, 14, 15), 1)]


def _qblocks(j):
    return [j, 7 - j, j + 8, 15 - j]


def _build(repeat=1, collective=True):
    nc = bacc.Bacc("TRN2", target_bir_lowering=False, debug=False, num_devices=8)

    xtq = nc.dram_tensor("xtq", [D, OWN], bf16, kind="ExternalInput").ap()
    xtv = nc.dram_tensor("xtv", [D, 512], bf16, kind="ExternalInput").ap()
    wqt = nc.dram_tensor("wqt", [D, H * HD], bf16, kind="ExternalInput").ap()
    wkvt = nc.dram_tensor("wkvt", [D, 2 * KV * HD], bf16,
                          kind="ExternalInput").ap()
    costv8 = nc.dram_tensor("costv8", [512, 512], bf16, kind="ExternalInput").ap()
    sintv8 = nc.dram_tensor("sintv8", [512, 512], bf16, kind="ExternalInput").ap()
    ident = nc.dram_tensor("ident", [P, P], f32, kind="ExternalInput").ap()
    wot = nc.dram_tensor("wot", [H * HD, D], bf16, kind="ExternalInput").ap()
    costq = nc.dram_tensor("costq", [P, OWN], bf16, kind="ExternalInput").ap()
    sintq = nc.dram_tensor("sintq", [P, OWN], bf16, kind="ExternalInput").ap()
    masku = nc.dram_tensor("masku", [NB, P, P], bf16, kind="ExternalInput").ap()
    negi = nc.dram_tensor("negi", [P, P], bf16, kind="ExternalInput").ap()
    onesr = nc.dram_tensor("onesr", [1, HD], f32, kind="ExternalInput").ap()
    y = nc.dram_tensor("y", [OWN, D], f32, kind="ExternalOutput").ap()

    def rope_full(dst, src, cosr, sinr, tmp_pool):
        """dst = bf16(src*cos + swap32(src)*sin_signed) over 128 rows.

        cosr rows r = cos[r%64]; sinr rows are sign-folded (-sin for
        (r%64)<32, +sin otherwise). The 32-row half-swaps run on the scalar
        engine (idle during projections); the wide elementwise ops run on
        DVE at full lane utilization."""
        xr = tmp_pool.tile([P, 512], bf16, tag="xrot", bufs=4, name="xr")
        for po in (0, 64):
            nc.scalar.copy(xr[po:po + 32, :], src[po + 32:po + 64, :])
            nc.scalar.copy(xr[po + 32:po + 64, :], src[po:po + 32, :])
        u = tmp_pool.tile([P, 512], bf16, tag="ropeu", bufs=4, name="u")
        v = tmp_pool.tile([P, 512], bf16, tag="ropev", bufs=4, name="v")
        nc.vector.tensor_mul(u[:], src[:], cosr)
        nc.vector.tensor_mul(v[:], xr[:], sinr)
        nc.vector.tensor_add(dst[:], u[:], v[:])

    with tile.TileContext(nc) as tc:
        _dpool_cm = tc.tile_pool(name="dram", bufs=1, space="DRAM")
        dpool = _dpool_cm.__enter__()
        _pers_cm = tc.tile_pool(name="pers", bufs=1)
        pers = _pers_cm.__enter__()

        agink = dpool.tile([AG_K], bf16, tag="agink", name="agink")
        aginv = dpool.tile([AG_V], bf16, tag="aginv", name="aginv")
        agoutk = dpool.tile([4, AG_K], bf16, tag="agoutk", name="agoutk")
        agoutv = dpool.tile([4, AG_V], bf16, tag="agoutv", name="agoutv")

        # persistent across stages
        cosq_t = pers.tile([P, OWN], bf16, tag="cosq_t", name="cosq_t")
        sinq_t = pers.tile([P, OWN], bf16, tag="sinq_t", name="sinq_t")
        ones_t = pers.tile([1, HD], f32r, tag="ones_t", name="ones_t")
        mask_t = pers.tile([P, NB * P], bf16, tag="mask_t", name="mask_t")
        negi_t = pers.tile([P, P], bf16, tag="negi_t", name="negi_t")
        qtr = [pers.tile([P, OWN], bf16, tag=f"qtr{i}", name=f"qtr{i}")
               for i in range(16)]
        # ktr_all[p, db*2048 + t] = k^T[128*db + p, t]
        ktr_all = pers.tile([P, 4 * T], bf16, tag="ktr_all", name="ktr_all")
        vaug = pers.tile([P, KV * VROW], bf16, tag="vaug", name="vaug")

        def emit_body():
         # ================= stages A-D: projections + AllGather ============
         with tc.tile_pool(name="consA", bufs=1) as cA, \
             tc.tile_pool(name="wk", bufs=1) as wkp, \
             tc.tile_pool(name="wst", bufs=4) as wst, \
             tc.tile_pool(name="xs", bufs=4) as xsp, \
             tc.tile_pool(name="tmp", bufs=4) as tmpp, \
             tc.tile_pool(name="cpy", bufs=3) as cpyp, \
             tc.tile_pool(name="pproj", bufs=1, space="PSUM") as pproj:

            # ---- stages A+B: token-sharded K and V projections ----
            costd = cA.tile([P, 4 * 512], bf16, tag="costd", name="costd")
            sintd = cA.tile([P, 4 * 512], bf16, tag="sintd", name="sintd")
            idtA = cA.tile([P, P], bf16, tag="idtA", name="idtA")
            idf = cA.tile([P, P], f32, tag="idf", name="idf")
            krT_f = cA.tile([P, 4 * 512], bf16, tag="krT_f", name="krT_f")

            psk = [pproj.tile([P, 512], f32, tag=f"k{i}", bufs=1,
                              name=f"psk{i}") for i in range(4)]
            psv = [pproj.tile([P, 512], f32, tag=f"v{i}", bufs=1,
                              name=f"psv{i}") for i in range(4)]
            xtq_sb = [wkp.tile([P, OWN], bf16, tag=f"xq{cb}", name=f"xq{cb}")
                      for cb in range(16)]
            wqf = [wkp.tile([P, H * HD], bf16, tag=f"wqf{cb}",
                            name=f"wqf{cb}") for cb in range(16)]
            for cb in range(16):
                wkv_t = wst.tile([P, 2 * KV * HD], bf16, tag="wkv",
                                 name="wkv")
                nc.sync.dma_start(wkv_t[:], wkvt[P * cb:P * cb + P, :])
                wk_t = wkv_t[:, 0:KV * HD]
                wv_t = wkv_t[:, KV * HD:2 * KV * HD]
                xv_t = xsp.tile([P, 512], bf16, tag="xtv", name="xtv")
                nc.sync.dma_start(
                    xv_t[:], xtv[P * cb:P * cb + P, :])
                # prefetch streams ride SWDGE (Pool is idle here) so the
                # hot KV loads keep HWDGE to themselves
                nc.gpsimd.dma_start(xtq_sb[cb][:], xtq[P * cb:P * cb + P, :])
                nc.sync.dma_start(wqf[cb][:], wqt[P * cb:P * cb + P, :])
                if cb < 4:
                    nc.gpsimd.dma_start(costd[:, 512 * cb:512 * cb + 512],
                                        costv8[P * cb:P * cb + P, :])
                    nc.gpsimd.dma_start(sintd[:, 512 * cb:512 * cb + 512],
                                        sintv8[P * cb:P * cb + P, :])
                elif cb == 4:
                    nc.gpsimd.dma_start(idf[:], ident[:])
                    nc.scalar.copy(idtA[:], idf[:])
                    nc.gpsimd.dma_start(negi_t[:], negi[:])
                    nc.gpsimd.dma_start(ones_t[:], onesr[:].bitcast(f32r))
                elif cb == 5:
                    nc.gpsimd.dma_start(cosq_t[:], costq[:])
                    nc.gpsimd.dma_start(sinq_t[:], sintq[:])
                elif cb == 6:
                    nc.gpsimd.dma_start(
                        mask_t.rearrange("p (b c) -> p b c", b=NB),
                        masku.rearrange("b p c -> p b c"))
                for vb in range(4):
                    nc.tensor.matmul(psk[vb][:],
                                     lhsT=xv_t[:, P * vb:P * vb + P],
                                     rhs=wk_t,
                                     start=(cb == 0), stop=(cb == 15))
                    nc.tensor.matmul(psv[vb][:],
                                     lhsT=xv_t[:, P * vb:P * vb + P],
                                     rhs=wv_t,
                                     start=(cb == 0), stop=(cb == 15))
            # V shard assembled in vaug layout locally (strided ACT copies +
            # ones columns), then one DMA out
            vs_big = cA.tile([P, VSH], bf16, tag="vs_big", name="vs_big")
            vsv = vs_big.rearrange("p (b k c) -> p b k c", k=KV, c=HD + 1)
            nc.vector.memset(vsv[:, :, :, HD], 1.0)
            for vb in range(4):
                nc.scalar.copy(
                    vsv[:, vb, :, 0:HD],
                    psv[vb].rearrange("p (k c) -> p k c", k=KV))
            nc.gpsimd.dma_start(
                aginv.rearrange("(p c) -> p c", p=P), vs_big[:])
            # K rope in (t, d) layout; transpose to (d, t); write shard
            for vb in range(4):
                cs = costd[:, 512 * vb:512 * vb + 512]
                sn = sintd[:, 512 * vb:512 * vb + 512]
                u = tmpp.tile([P, 512], f32, tag="ropeu", bufs=4, name="u")
                nc.vector.tensor_mul(u[:], psk[vb][:], cs)
                vv = tmpp.tile([P, 512], f32, tag="ropev", bufs=4, name="vv")
                pr = psk[vb].rearrange("p (h s w) -> p h s w", s=2, w=32)
                vr = vv.rearrange("p (h s w) -> p h s w", s=2, w=32)
                sr = sn.rearrange("p (h s w) -> p h s w", s=2, w=32)
                nc.vector.tensor_mul(vr[:, :, 0, :], pr[:, :, 1, :],
                                     sr[:, :, 0, :])
                nc.vector.tensor_mul(vr[:, :, 1, :], pr[:, :, 0, :],
                                     sr[:, :, 1, :])
                kr = cpyp.tile([P, 512], bf16, tag="kr", name="kr")
                nc.vector.tensor_add(kr[:], u[:], vv[:])
                for db in range(4):
                    ptr = pproj.tile([P, P], bf16, tag=f"v{db}", bufs=1,
                                     name="ptrA")
                    nc.tensor.transpose(ptr[:], kr[:, P * db:P * db + P],
                                        idtA[:])
                    dst = krT_f[:, 512 * db + P * vb:512 * db + P * vb + P]
                    if db % 2 == 0:
                        nc.vector.tensor_copy(dst, ptr[:])
                    else:
                        nc.scalar.copy(dst, ptr[:])
            nc.gpsimd.dma_start(
                agink.rearrange("(p c) -> p c", p=P), krT_f[:])

            # ---- stage C: split AllGathers (V first — ready earlier) ----
            if collective:
                nc.gpsimd.collective_compute(
                    "AllGather",
                    mybir.AluOpType.bypass,
                    replica_groups=[[0, 1, 2, 3], [4, 5, 6, 7]],
                    ins=[aginv.opt()],
                    outs=[agoutv.opt()],
                )
                nc.gpsimd.collective_compute(
                    "AllGather",
                    mybir.AluOpType.bypass,
                    replica_groups=[[0, 1, 2, 3], [4, 5, 6, 7]],
                    ins=[agink.opt()],
                    outs=[agoutk.opt()],
                )
            else:
                for g in range(4):
                    nc.gpsimd.dma_start(
                        agoutv[g].rearrange("(a b) -> a b", b=8320),
                        aginv.rearrange("(a b) -> a b", b=8320))
                for g in range(4):
                    nc.gpsimd.dma_start(
                        agoutk[g].rearrange("(a b) -> a b", b=8192),
                        agink.rearrange("(a b) -> a b", b=8192))

            # ---- unpack: both in-gather layouts need 4 DMAs each ----
            for g in range(4):
                ksrc = agoutk[g, 0:AG_K].rearrange(
                    "(p d t) -> p d t", p=P, t=512)
                kdst = ktr_all.rearrange("p (d t) -> p d t", d=4)[
                    :, :, 512 * g:512 * g + 512]
                nc.gpsimd.dma_start(kdst, ksrc)
            for g in range(4):
                nc.gpsimd.dma_start(
                    vaug[:, VSH * g:VSH * g + VSH],
                    agoutv[g, 0:AG_V].rearrange("(p c) -> p c", p=P))

            # ---- stage D: Q projection (owned tokens) + rope ----
            for qg in range(4):
                tg = "k" if qg % 2 == 0 else "v"
                psq = [pproj.tile([P, 512], f32, tag=f"{tg}{i}", bufs=1,
                                  name=f"psq{i}") for i in range(4)]
                for cb in range(16):
                    wq_t = wqf[cb][:, 512 * qg:512 * qg + 512]
                    for qi in range(4):
                        nc.tensor.matmul(
                            psq[qi][:], lhsT=wq_t[:, P * qi:P * qi + P],
                            rhs=xtq_sb[cb], start=(cb == 0), stop=(cb == 15))
                for qi in range(4):
                    rope_full(qtr[4 * qg + qi], psq[qi], cosq_t[:],
                              sinq_t[:], tmpp)

         # ================= stages E-G: attention + fused o_proj ===========
         with tc.tile_pool(name="psS2", bufs=1, space="PSUM") as psS2, \
             tc.tile_pool(name="psS1", bufs=1, space="PSUM") as psS1, \
             tc.tile_pool(name="psO", bufs=1, space="PSUM") as psO, \
             tc.tile_pool(name="ptp", bufs=4) as ptp, \
             tc.tile_pool(name="nrm", bufs=3) as nrm, \
             tc.tile_pool(name="wos", bufs=1) as wos, \
             tc.tile_pool(name="otrg", bufs=1) as otrp, \
             tc.tile_pool(name="ysbp", bufs=1) as ysbp, \
             tc.tile_pool(name="yc", bufs=2) as ycp:

            otr = [otrp.tile([P, OWN], bf16, tag=f"otr{i % 8}",
                             name=f"otr{i}") for i in range(16)]
            ysb = {}
            for eb in range(4):
                for tb in range(4):
                    ysb[(eb, tb)] = ysbp.tile(
                        [P, 512], bf16, tag=f"ysb{eb}_{tb}",
                        name=f"ysb{eb}_{tb}")

            # wo chunk prefetch: o_proj contraction chunk ci covers
            # ab in CHUNKS[ci]; prefetched two chunks ahead.
            CHUNKS = [(0, 4), (4, 8), (8, 12), (12, 14), (14, 16)]
            wo_ch = {}

            def prefetch_chunk(ci):
                for ab in range(*CHUNKS[ci]):
                    for eb in range(4):
                        wt = wos.tile([P, 512], bf16, tag=f"wo{ab % 8}_{eb}",
                                      name=f"wo{ab}_{eb}", bufs=1)
                        nc.sync.dma_start(
                            wt[:], wot[P * ab:P * ab + P,
                                       512 * eb:512 * eb + 512])
                        wo_ch[(eb, ab)] = wt

            prefetch_chunk(0)
            prefetch_chunk(1)

            # o_proj work queue: (q, eb, tb) quanta emitted between attention
            # groups to fill the PE; the PSUM slot tag is picked at pop time
            # (the score tags join the rotation once attention has drained).
            oproj_work = []

            def enqueue_chunk(ci):
                for eb in range(4):
                    for tb in range(4):
                        oproj_work.append((ci, eb, tb))

            def emit_oproj(ci, eb, tb, tag):
                lo, hi = CHUNKS[ci]
                pool = psO if tag == "oaug" else psS2
                psg = pool.tile([P, 512], f32, tag=tag,
                                bufs=(2 if tag == "oaug" else 3), name="psg")
                for ab in range(lo, hi):
                    nc.tensor.matmul(
                        psg[:], lhsT=otr[ab][:, P * tb:P * tb + P],
                        rhs=wo_ch[(eb, ab)],
                        start=(ab == lo), stop=(ab == hi - 1))
                t = ysb[(eb, tb)]
                if ci == 0:
                    nc.vector.tensor_copy(t[:], psg[:])
                elif ci < len(CHUNKS) - 1:
                    nc.vector.tensor_add(t[:], t[:], psg[:])
                else:
                    yt = ycp.tile([P, 512], f32, tag="yt", bufs=6,
                                  name="yt")
                    nc.vector.tensor_add(yt[:], t[:], psg[:])
                    eng = nc.sync if (eb + tb) % 2 == 0 else nc.gpsimd
                    eng.dma_start(
                        y[P * tb:P * tb + P, 512 * eb:512 * eb + 512], yt[:])

            def pop_oproj(k, tags=("oaug",)):
                for i in range(min(k, len(oproj_work))):
                    ci, eb, tb = oproj_work.pop(0)
                    emit_oproj(ci, eb, tb, tags[i % len(tags)])

            def emit_group(st, pt, kbs, kind, w, v4, m, side):
                qs = P * (kbs[0] // 4)
                n = OWN - qs
                kpo = HD * side
                kt = ktr_all[kpo:kpo + HD,
                             2048 * v4:2048 * v4 + 2048]
                rhs = qtr[4 * v4 + m][kpo:kpo + HD, qs:OWN]
                for i, kb in enumerate(kbs):
                    nc.tensor.matmul(
                        st[:, w * i:w * i + n],
                        lhsT=kt[:, P * kb:P * kb + P],
                        rhs=rhs,
                        start=(kind == 2 or i == 0), stop=False,
                        skip_group_check=True)
                if len(kbs) == 4:
                    nc.tensor.matmul(
                        st[:, 0:512], lhsT=negi_t[:],
                        rhs=mask_t[:, P * kbs[0]:P * kbs[0] + 512],
                        start=False, stop=True, skip_group_check=True)
                else:
                    for i, kb in enumerate(kbs):
                        nc.tensor.matmul(
                            st[:, w * i:w * i + P], lhsT=negi_t[:],
                            rhs=mask_t[:, P * kb:P * kb + P],
                            start=False, stop=(i == len(kbs) - 1),
                            skip_group_check=True)
                # one exp per group (strided across banks for kind 2)
                if kind == 2 and n < 512:
                    stv = st.rearrange("p (s c) -> p s c", s=2)
                    ptv = pt.rearrange("p (s c) -> p s c", s=2)
                    nc.scalar.activation(ptv[:, :, 0:n], stv[:, :, 0:n],
                                         Exp, scale=0.125)
                else:
                    m = w * (len(kbs) - 1) + n
                    nc.scalar.activation(pt[:, 0:m], st[:, 0:m], Exp,
                                         scale=0.125)

            for uu in range(16):
                v4 = uu // 4
                # head pair interleaved at group granularity: tile uu rows
                # 0:64 = head a (kv 2*v4), rows 64:128 = head a+4 (2*v4+1)
                m_u = uu % 4
                oaug = [psO.tile([P, OWN], f32, tag="oaug", bufs=2,
                                 name=f"oaug{s_}") for s_ in range(2)]
                pend = {0: [], 1: []}  # per side, AV lag 2
                for kbs, kind in GROUPS:
                    for side in range(2):
                        kh = 2 * v4 + side
                        if kind == 2:
                            st = psS2.tile([P, 1024], f32, tag="st2",
                                           bufs=2, name="st2")
                            w = 512
                        else:
                            st = psS1.tile([P, 512], f32, tag="st1",
                                           bufs=2, name="st1")
                            w = 512 // len(kbs)
                        pt = ptp.tile([P, 1024], bf16, tag="pt", bufs=10,
                                      name="pt")
                        emit_group(st, pt, kbs, kind, w, v4, m_u, side)
                        pend[side].append((pt, kbs, w))
                        if len(pend[side]) > 2:
                            _emit_avs(nc, pend[side].pop(0), vaug,
                                      oaug[side], kh)
                for side in range(2):
                    for pe_ in pend[side]:
                        _emit_avs(nc, pe_, vaug, oaug[side], 2 * v4 + side)
                for side in range(2):
                    # normalization: rec = 1/sums; Pool broadcasts to 64 rows
                    rec = nrm.tile([1, OWN], f32, tag="rec", name="rec")
                    nc.vector.reciprocal(rec[:], oaug[side][HD:HD + 1, :])
                    pbs = nrm.tile([HD, OWN], f32, tag="pbs", bufs=3,
                                   name="pbs")
                    nc.gpsimd.partition_broadcast(pbs[:], rec[:])
                    nc.vector.tensor_mul(
                        otr[uu][HD * side:HD * side + HD, :],
                        oaug[side][0:HD, :], pbs[:])
                pop_oproj(4, tags=("st1",))
                ci = {3: 0, 7: 1, 11: 2, 13: 3, 15: 4}.get(uu)
                if ci is not None:
                    enqueue_chunk(ci)
                    if ci + 2 < len(CHUNKS):
                        prefetch_chunk(ci + 2)
            pop_oproj(len(oproj_work), tags=("oaug", "st1", "st2"))

        for _rep in range(repeat):
            emit_body()

        _pers_cm.__exit__(None, None, None)
        _dpool_cm.__exit__(None, None, None)

    nc.compile()
    return nc


def _emit_avs(nc, pend, vaug, oaug, kh):
    pt, kbs, w = pend
    qs = P * (kbs[0] // 4)
    n = OWN - qs
    for i, kb in enumerate(kbs):
        base = (HD + 1) * (KV * kb + kh)
        nc.tensor.matmul(
            oaug[0:HD + 1, qs:OWN],
            lhsT=vaug[:, base:base + HD + 1],
            rhs=pt[:, w * i:w * i + n],
            start=(kb == 0), stop=(kb == NB - 1))


_NC = None


def _get_nc():
    global _NC
    if _NC is None:
        _NC = _build()
    return _NC


def _head_perm():
    """Pair each even-kv head with its odd-kv partner (+4) in one 128-dim
    block, so q partition parity matches the kv head parity in kT tiles."""
    order = []
    for u in range(16):
        a = 8 * (u // 4) + u % 4
        for h in (a, a + 4):
            order.extend(range(HD * h, HD * h + HD))
    return np.asarray(order)


def _in_maps(x, cos, sin, Wq, Wk, Wv, Wo):
    xT = np.ascontiguousarray(np.transpose(np.asarray(x, np.float32), (0, 2, 1)))
    perm = _head_perm()
    import ml_dtypes as _mld
    WqT = np.ascontiguousarray(
        np.asarray(Wq, np.float32).T[:, perm].astype(_mld.bfloat16))
    WkvT = np.ascontiguousarray(np.concatenate(
        [np.asarray(Wk, np.float32).T, np.asarray(Wv, np.float32).T],
        axis=1).astype(_mld.bfloat16))
    WoT = np.ascontiguousarray(
        np.asarray(Wo, np.float32).T[perm, :].astype(_mld.bfloat16))
    cosT = np.asarray(cos, np.float32).T        # (64, T)
    sinT = np.asarray(sin, np.float32).T
    # 128-row rope tables: row r uses hd-dim r%64; sin rows sign-folded
    # (-sin for (r%64)<32) so rope is x*cos2 + swap32(x)*sin2 on 128 rows.
    sgn = np.where(np.arange(HD) < HD // 2, -1.0, 1.0).astype(np.float32)
    sgnc = sgn  # same sign-fold along the free axis for (t, d) layout
    cos2 = np.ascontiguousarray(np.tile(cosT, (2, 1)))              # (128, T)
    sin2 = np.ascontiguousarray(np.tile(sinT * sgn[:, None], (2, 1)))
    ones = np.ones((1, HD), np.float32)
    maps = []
    for c in range(8):
        b, j = c // 4, c % 4
        qb = _qblocks(j)
        cols = np.concatenate([np.arange(P * g, P * g + P) for g in qb])
        mask = np.empty((NB, P, P), _mld.bfloat16)
        ki = np.arange(P)[:, None]
        qi = np.arange(P)[None, :]
        for kb in range(NB):
            qg = qb[kb // 4]
            mask[kb] = np.where(P * kb + ki <= P * qg + qi, 0.0, 1.0)
        negi_np = (np.eye(P, dtype=np.float32) *
                   np.float32(-2.0 ** 30)).astype(_mld.bfloat16)
        maps.append({
            "xtq": np.ascontiguousarray(
                xT[b][:, cols].astype(_mld.bfloat16)),
            "xtv": np.ascontiguousarray(
                xT[b][:, 512 * j:512 * j + 512].astype(_mld.bfloat16)),
            "wqt": WqT,
            "wkvt": WkvT,
            "wot": WoT,
            "costq": np.ascontiguousarray(cos2[:, cols]).astype(
                _mld.bfloat16),
            "sintq": np.ascontiguousarray(sin2[:, cols]).astype(
                _mld.bfloat16),
            "costv8": np.ascontiguousarray(
                np.tile(cosT.T[512 * j:512 * j + 512, :],
                        (1, KV))).astype(_mld.bfloat16),
            "sintv8": np.ascontiguousarray(
                np.tile(sinT.T[512 * j:512 * j + 512, :] * sgnc[None, :],
                        (1, KV))).astype(_mld.bfloat16),
            "ident": np.eye(P, dtype=np.float32),
            "masku": mask,
            "negi": negi_np,
            "onesr": ones,
        })
    return maps


def kernel(x, cos, sin, Wq, Wk, Wv, Wo):
    nc = _get_nc()
    maps = _in_maps(x, cos, sin, Wq, Wk, Wv, Wo)
    res = run_bass_kernel_spmd(nc, maps, list(range(8)))
    out = np.empty((B, T, D), np.float32)
    for c in range(8):
        b, j = c // 4, c % 4
        yc = res.results[c]["y"]
        for s, qg in enumerate(_qblocks(j)):
            out[b, P * qg:P * qg + P, :] = yc[P * s:P * s + P, :]
    return out
